# revision 7
# baseline (speedup 1.0000x reference)
"""Deformable-conv (depth-aware) Trainium2 kernel.

Sharding: pure data parallel — 8 cores = 2 images x 4 H-strips of 32 rows.

Device algorithm per core (strip of 32 rows x 128 cols = 4096 pixels, 9
samples each):
  1. offset conv (PE): off[pix, 18] = sum_k x_slice @ w_p_k   (K=65 incl bias)
  2. pass-1 depth bilinear sampling computed DENSELY (no gather): the
     offsets are < 1 in magnitude, so each sample's 2x2 bilinear footprint
     lies in a 3x3 window around its integer base position.  The depth map
     is host-prepped into 5 column-shifted clamp-extended tiles, and the
     sampling is a separable (3 row-weights x 3 col-weights) accumulation
     of shifted views — all on-chip, zero DMA.  Depth weights dw, m (ACT exp).
  3. off2 = off * dw; pass-2 coords/weights; per-corner weights w4 = m*row*col
  4. dma_gather of 2x2x64ch x-records (fp16, channel-major/corner-minor),
     one DVE mul (weights broadcast over channels) + corner-reduce
  5. DMA-transpose to [(n,c), pix] tiles, PE matmul vs w_conv -> out strip
"""
import numpy as np

B, C, H, W = 2, 64, 128, 128
N = 9
WP = W + 2           # 130 padded width
SH = 4               # coordinate shift: keeps sample coords positive so
                     # int-cast truncation == floor (no correction ops)
WP2 = WP + 2 * SH    # 138 shifted table width
SP = H // 4          # 32 strip rows
NPIX = SP * W        # 4096 pixels per strip
NS = NPIX * N        # 36864 samples per strip
NREC = WP2 * WP2     # shifted-table records
OCLIP = 0.99951171875  # fp16-exact clamp keeping pass-1 window in 3x3

_CACHE = {}


# ---------------------------------------------------------------------------
# device program
# ---------------------------------------------------------------------------
def _build_program():
    import concourse.bacc as bacc
    import concourse.tile as tile
    import concourse.mybir as mybir

    dt = mybir.dt
    Alu = mybir.AluOpType
    Act = mybir.ActivationFunctionType

    nc = bacc.Bacc("TRN2", target_bir_lowering=False, debug=False,
                   enable_asserts=False, num_devices=8)

    xs_d = nc.dram_tensor("xs", [65, 34 * WP], dt.float16, kind="ExternalInput")
    r2_d = nc.dram_tensor("r2", [NREC, 256], dt.float16, kind="ExternalInput")
    det_d = nc.dram_tensor("det", [128, 5 * 36], dt.float32, kind="ExternalInput")
    base_d = nc.dram_tensor("base", [128, 32 * 18], dt.float32, kind="ExternalInput")
    dcen_d = nc.dram_tensor("dcen", [128, 32], dt.float32, kind="ExternalInput")
    wp_d = nc.dram_tensor("wp", [65, 9 * 18], dt.float16, kind="ExternalInput")
    w2_d = nc.dram_tensor("w2", [128, 5 * 64], dt.float16, kind="ExternalInput")
    out_d = nc.dram_tensor("o", [64, NPIX], dt.float16, kind="ExternalOutput")

    import os
    H1 = int(os.environ.get('KSTG', '16'))  # rows per pipeline stage
    RC = int(os.environ.get('KRC', '8'))    # rows per gather
    BC = int(os.environ.get('KBC', '2'))    # rows per blend/matmul chunk
    OB = int(os.environ.get('KOB', '4'))    # blend chunks per output store
    KA2 = int(os.environ.get('KA2', '2'))   # add2: 0=DVE 1=Pool 2=alt
    KDP = int(os.environ.get('KDP', '3'))   # dense-pass1: every KDP'th tt op on Pool (0=none)

    with tile.TileContext(nc) as tc:
        with (
            tc.tile_pool(name="const", bufs=1) as cp,
            tc.tile_pool(name="work", bufs=2) as wk,
            tc.tile_pool(name="g2p", bufs=int(os.environ.get("KG2B", "2"))) as g2p,
            tc.tile_pool(name="pstp", bufs=4, space="PSUM") as pstp,
            tc.tile_pool(name="urp", bufs=int(os.environ.get("KURB", "2"))) as urp,
            tc.tile_pool(name="xtp", bufs=int(os.environ.get("KXTB", "2"))) as xtp,
            tc.tile_pool(name="osp", bufs=2) as osp,
            tc.tile_pool(name="psc", bufs=2, space="PSUM") as psc,
            tc.tile_pool(name="psm", bufs=2, space="PSUM") as psm,
        ):
            f32 = dt.float32
            # ---- constants
            xs = cp.tile([65, 34, WP], dt.float16, tag="xs")
            xsv = xs_d[:].rearrange("c (a b) -> c a b", b=WP)
            nc.sync.dma_start(xs[:, 0:18, :], xsv[:, 0:18, :])
            nc.sync.dma_start(xs[:, 18:34, :], xsv[:, 18:34, :])
            det = cp.tile([128, 5, 36], f32, tag="det")
            nc.sync.dma_start(det[:], det_d[:].rearrange("p (a b) -> p a b", b=36))
            base = cp.tile([128, 32, 18], f32, tag="base")
            nc.sync.dma_start(base[:], base_d[:].rearrange("p (a b) -> p a b", b=18))
            dcen = cp.tile([128, 32], f32, tag="dcen")
            nc.sync.dma_start(dcen[:], dcen_d[:])
            wp = cp.tile([65, 9 * 18], dt.float16, tag="wp")
            nc.sync.dma_start(wp[:], wp_d[:])
            w2 = cp.tile([128, 5 * 64], dt.float16, tag="w2")
            nc.sync.dma_start(w2[:], w2_d[:])
            ident = cp.tile([128, 128], dt.float16, tag="ident")
            from concourse.masks import make_identity
            make_identity(nc, ident[:])

            # round-robin engine picker for dense-pass1 tensor_tensor ops
            _dp_ct = [0]

            def dpeng():
                _dp_ct[0] += 1
                if KDP and _dp_ct[0] % KDP == 0:
                    return nc.gpsimd
                return nc.vector

            # pass-2 weight-math engine split: every KCL'th op on Pool
            KCL = int(os.environ.get('KCL', '2'))
            _cl_ct = [0]

            def cleng():
                _cl_ct[0] += 1
                if KCL and _cl_ct[0] % KCL == 0:
                    return nc.gpsimd
                return nc.vector

            def sample_floor(Pc, bound, RR, pool, pfx):
                """floor/clip in SH-shifted coords (pass-2 path, as baseline)."""
                fi = pool.tile([128, RR, 18], dt.int32, tag=pfx + "sm_fi")
                nc.scalar.activation(fi[:], Pc[:], Act.Copy, bias=-0.5)
                f = pool.tile([128, RR, 18], f32, tag=pfx + "sm_f")
                nc.scalar.copy(f[:], fi[:])
                qlt = pool.tile([128, RR, 18], f32, tag=pfx + "sm_qlt")
                nc.vector.tensor_scalar(qlt[:], f[:], float(SH), float(bound - 1 + SH),
                                        Alu.max, Alu.min)
                qrb = pool.tile([128, RR, 18], f32, tag=pfx + "sm_qrb")
                nc.vector.tensor_scalar(qrb[:], f[:], float(SH - 1), float(bound - 2 + SH),
                                        Alu.max, Alu.min)
                nc.scalar.add(qrb[:], qrb[:], 1.0)
                r0 = pool.tile([128, RR, 18], f32, tag=pfx + "sm_r0")
                nc.vector.tensor_scalar(r0[:], qlt[:], float(SH), float(bound - 2 + SH),
                                        Alu.max, Alu.min)
                return r0, qlt, qrb

            def sample_weights(Pc, bound, r0, qlt, qrb, RR, pool, pfx):
                pc = pool.tile([128, RR, 18], f32, tag=pfx + "sm_pc")
                nc.vector.tensor_scalar(pc[:], Pc[:], float(SH), float(bound - 1 + SH),
                                        Alu.max, Alu.min)
                gl = pool.tile([128, RR, 18], f32, tag=pfx + "sm_gl")
                nc.vector.scalar_tensor_tensor(gl[:], qlt[:], 1.0, pc[:], Alu.add, Alu.subtract)
                gr = pool.tile([128, RR, 18], f32, tag=pfx + "sm_gr")
                nc.vector.scalar_tensor_tensor(gr[:], pc[:], 1.0, qrb[:], Alu.add, Alu.subtract)
                eq = pool.tile([128, RR, 18], f32, tag=pfx + "sm_eq")
                wA = pool.tile([128, RR, 18], f32, tag=pfx + "sm_wA")
                wB = pool.tile([128, RR, 18], f32, tag=pfx + "sm_wB")
                tmp = pool.tile([128, RR, 18], f32, tag=pfx + "sm_tmp")
                cleng().tensor_tensor(eq[:], qlt[:], r0[:], Alu.is_equal)
                cleng().tensor_tensor(wA[:], gl[:], eq[:], Alu.mult)
                cleng().tensor_tensor(eq[:], qrb[:], r0[:], Alu.is_equal)
                cleng().tensor_tensor(tmp[:], gr[:], eq[:], Alu.mult)
                cleng().tensor_tensor(wA[:], wA[:], tmp[:], Alu.add)
                nc.vector.scalar_tensor_tensor(
                    eq[:], qlt[:], -1.0, r0[:], Alu.add, Alu.is_equal)
                cleng().tensor_tensor(wB[:], gl[:], eq[:], Alu.mult)
                nc.vector.scalar_tensor_tensor(
                    eq[:], qrb[:], -1.0, r0[:], Alu.add, Alu.is_equal)
                cleng().tensor_tensor(tmp[:], gr[:], eq[:], Alu.mult)
                cleng().tensor_tensor(wB[:], wB[:], tmp[:], Alu.add)
                return wA, wB

            def make_idx(r0, name, RR, pool):
                idxf = pool.tile([128, RR, 9], f32, tag=name + "_f")
                nc.vector.scalar_tensor_tensor(
                    idxf[:], r0[:, :, 0:9], float(WP2), r0[:, :, 9:18],
                    Alu.mult, Alu.add)
                idxi = pool.tile([128, RR * 9], dt.int16, tag=name + "_i")
                nc.vector.tensor_copy(idxi[:], idxf[:].rearrange("p a b -> p (a b)"))
                idxw = pool.tile([128, RR * 9, 8], dt.int16, tag=name + "_w")
                for s in range(8):
                    nc.sync.dma_start(idxw[0:16, :, s], idxi[16 * s:16 * (s + 1), :])
                nc.sync.dma_start(idxw[16:32, :, :], idxw[0:16, :, :])
                nc.sync.dma_start(idxw[32:64, :, :], idxw[0:32, :, :])
                nc.sync.dma_start(idxw[64:96, :, :], idxw[0:32, :, :])
                nc.sync.dma_start(idxw[96:128, :, :], idxw[0:32, :, :])
                return idxw

            # ---------------- per-stage emission closures ----------------
            def emit_A(rs, nr):
                """offset conv rows [rs, rs+nr) -> OFF [128, nr, 18] (PE)."""
                OFF = wk.tile([128, nr, 18], f32, tag="OFF")
                for bg in range(nr // 4):
                    ps = psc.tile([128, 72], f32)
                    for bb in range(4):
                        b = rs + bg * 4 + bb
                        for k in range(9):
                            drr, dcc = k // 3, k % 3
                            nc.tensor.matmul(
                                ps[:, bb * 18:(bb + 1) * 18],
                                lhsT=xs[:, b + drr, dcc:dcc + 128],
                                rhs=wp[:, k * 18:(k + 1) * 18],
                                start=(k == 0), stop=(k == 8),
                            )
                    nc.scalar.copy(OFF[:, bg * 4:(bg + 1) * 4, :],
                                   ps[:].rearrange("p (a b) -> p a b", b=18))
                return OFF

            def emit_B_dense(rs, nr, OFF):
                """pass-1 depth sampling, dense 3x3 separable form (no DMA).

                Returns dwe, mm [128, nr, 9]."""
                offc = wk.tile([128, nr, 18], f32, tag="b_offc")
                nc.vector.tensor_scalar(offc[:], OFF[:], -OCLIP, OCLIP,
                                        Alu.max, Alu.min)
                P1 = wk.tile([128, nr, 18], f32, tag="b_P1")
                nc.vector.tensor_add(P1[:], offc[:], base[:, rs:rs + nr, :])
                fi = wk.tile([128, nr, 18], dt.int32, tag="b_fi")
                nc.scalar.activation(fi[:], P1[:], Act.Copy, bias=-0.5)
                f = wk.tile([128, nr, 18], f32, tag="b_f")
                nc.scalar.copy(f[:], fi[:])
                q0 = wk.tile([128, nr, 18], f32, tag="b_q0")
                nc.vector.tensor_scalar(q0[:], f[:], float(SH), float(H - 1 + SH),
                                        Alu.max, Alu.min)
                q1c = wk.tile([128, nr, 18], f32, tag="b_q1c")
                nc.vector.tensor_scalar(q1c[:], f[:], float(SH - 1), float(H - 2 + SH),
                                        Alu.max, Alu.min)
                pc = wk.tile([128, nr, 18], f32, tag="b_pc")
                nc.vector.tensor_scalar(pc[:], P1[:], float(SH), float(H - 1 + SH),
                                        Alu.max, Alu.min)
                g0 = wk.tile([128, nr, 18], f32, tag="b_g0")
                nc.vector.scalar_tensor_tensor(g0[:], q0[:], 1.0, pc[:],
                                               Alu.add, Alu.subtract)
                g1 = wk.tile([128, nr, 18], f32, tag="b_g1")
                dpeng().tensor_sub(g1[:], pc[:], q1c[:])
                mA = wk.tile([128, nr, 18], f32, tag="b_mA")
                nc.vector.scalar_tensor_tensor(mA[:], f[:], 1.0, base[:, rs:rs + nr, :],
                                               Alu.add, Alu.is_equal)
                # W3 components: Wm = mA*g0, W0 = g0 + mA*(g1-g0), Wp = g1 - mA*g1
                d = wk.tile([128, nr, 18], f32, tag="b_d")
                dpeng().tensor_sub(d[:], g1[:], g0[:])
                Wm = wk.tile([128, nr, 18], f32, tag="b_Wm")
                dpeng().tensor_mul(Wm[:], mA[:], g0[:])
                t = wk.tile([128, nr, 18], f32, tag="b_t")
                dpeng().tensor_mul(t[:], mA[:], d[:])
                W0 = wk.tile([128, nr, 18], f32, tag="b_W0")
                dpeng().tensor_add(W0[:], g0[:], t[:])
                dpeng().tensor_mul(t[:], mA[:], g1[:])
                Wp = wk.tile([128, nr, 18], f32, tag="b_Wp")
                dpeng().tensor_sub(Wp[:], g1[:], t[:])
                W3 = (Wm, W0, Wp)
                # separable accumulation over the 3x3 window
                V = wk.tile([128, nr, 9], f32, tag="b_V")
                CI = wk.tile([128, nr, 9], f32, tag="b_CI")
                tt = wk.tile([128, nr, 9], f32, tag="b_tt")
                da = det[:]
                for ai in range(3):          # row window offset a'' = ai-1
                    for bi in range(3):      # col window offset b'' = bi-1
                        # DET view: dims (i: stride 1, nr) (dr: stride 1, 3)
                        # (dc: stride 36, 3); offset = bi*36 + rs + ai
                        dv = da.__replace__(
                            offset=da.offset + bi * 36 + rs + ai,
                            ap=type(da.ap)(
                                [[180, 128], [1, nr], [1, 3], [36, 3]]))
                        tgt = CI if bi == 0 else tt
                        dpeng().tensor_tensor(
                            tgt[:].rearrange("p a (u v) -> p a u v", u=3),
                            W3[bi][:, :, 9:18].rearrange("p a (u v) -> p a u v", u=3),
                            dv, Alu.mult)
                        if bi > 0:
                            dpeng().tensor_add(CI[:], CI[:], tt[:])
                    tgt = V if ai == 0 else tt
                    dpeng().tensor_mul(tgt[:], W3[ai][:, :, 0:9], CI[:])
                    if ai > 0:
                        nc.vector.tensor_add(V[:], V[:], tt[:])
                dd = wk.tile([128, nr, 9], f32, tag="b_dd")
                dwe = wk.tile([128, nr, 9], f32, tag="b_dwe")
                mm = wk.tile([128, nr, 9], f32, tag="b_mm")
                nc.vector.tensor_sub(
                    dd[:], dcen[:, rs:rs + nr, None].to_broadcast((128, nr, 9)),
                    V[:])
                nc.scalar.activation(dd[:], dd[:], Act.Abs)
                nc.scalar.activation(dwe[:], dd[:], Act.Exp, scale=-4.0)
                nc.scalar.activation(mm[:], dd[:], Act.Exp, scale=-1.0)
                return dwe, mm

            def emit_C(rs, nr, OFF, dwe, mm):
                NRW = nr * 9
                P2 = wk.tile([128, nr, 18], f32, tag="P2")
                nc.vector.scalar_tensor_tensor(
                    P2[:, :, 0:9], dwe[:], 0.25, OFF[:, :, 0:9], Alu.add, Alu.mult)
                nc.vector.scalar_tensor_tensor(
                    P2[:, :, 9:18], dwe[:], 0.25, OFF[:, :, 9:18], Alu.add, Alu.mult)
                nc.vector.tensor_add(P2[:], P2[:], base[:, rs:rs + nr, :])
                r0_2, qlt2, qrb2 = sample_floor(P2, H + 2, nr, wk, "c")
                idx2w = make_idx(r0_2, "idx2", nr, wk)
                wA2, wB2 = sample_weights(P2, H + 2, r0_2, qlt2, qrb2, nr, wk, "c")
                wTm = wk.tile([128, nr, 9], f32, tag="wTm")
                nc.vector.tensor_mul(wTm[:], wA2[:, :, 0:9], mm[:])
                wBm = wk.tile([128, nr, 9], f32, tag="wBm")
                nc.vector.tensor_mul(wBm[:], wB2[:, :, 0:9], mm[:])
                w4 = wk.tile([128, NRW, 4], f32, tag="w4")
                w4v = w4[:].rearrange("p (a b) c -> p a b c", b=9)
                nc.vector.tensor_mul(w4v[:, :, :, 0], wTm[:], wA2[:, :, 9:18])
                nc.vector.tensor_mul(w4v[:, :, :, 1], wTm[:], wB2[:, :, 9:18])
                nc.vector.tensor_mul(w4v[:, :, :, 2], wBm[:], wA2[:, :, 9:18])
                nc.vector.tensor_mul(w4v[:, :, :, 3], wBm[:], wB2[:, :, 9:18])
                w4h2 = wk.tile([128, NRW, 4, 2], dt.float16, tag="w4h2")
                nc.scalar.copy(
                    w4h2[:], w4[:, :, :, None].to_broadcast((128, NRW, 4, 2)))
                return idx2w, w4h2

            R9G = RC * 9   # gather slots per partition per gather
            R9 = BC * 9    # blend slots per partition per chunk

            def emit_D_trig(g, idx2w):
                g2 = g2p.tile([128, R9G, 256], dt.float16)
                nc.gpsimd.dma_gather(
                    out_ap=g2[:], in_ap=r2_d[:],
                    idxs_ap=idx2w[:, R9G * g:R9G * (g + 1), :],
                    num_idxs=1152 * RC, num_idxs_reg=1152 * RC, elem_size=256,
                    single_packet=False)
                return g2

            def emit_D_blend(c, g2, cg, w4h2):
                # blend in place: g2 is dead after the corner adds
                g2s = g2[:, R9 * cg:R9 * (cg + 1), :]
                u4 = g2s.rearrange("p a (h k l) -> p a h k l", k=4, l=2)
                nc.vector.tensor_tensor(
                    u4, u4,
                    w4h2[:, R9 * c:R9 * (c + 1), None, :, :].to_broadcast(
                        (128, R9, 32, 4, 2)),
                    Alu.mult)
                u4v = g2s.rearrange("p a (h k l) -> p (a h) k l", k=4, l=2)
                nc.vector.tensor_tensor(u4v[:, :, 0:2, :], u4v[:, :, 0:2, :],
                                        u4v[:, :, 2:4, :], Alu.add)
                ur = urp.tile([128, BC * 576 + 64], dt.float16)
                nc.vector.memset(ur[:, BC * 576:BC * 576 + 64], 0.0)
                urv = ur[:, 0:BC * 576].rearrange("p (a l) -> p a l", l=2)
                eng = (nc.gpsimd if (KA2 == 1 or (KA2 == 2 and c % 2 == 0))
                       else nc.vector)
                eng.tensor_tensor(urv, u4v[:, :, 0, :], u4v[:, :, 1, :],
                                  Alu.add)
                return ur

            def emit_D_mm(rs, c, ur, osb):
                xt = xtp.tile([128, 5, BC * 128], dt.float16)
                for bb in range(BC):
                    # 5 transposes land in one PSUM bank -> single Act copy
                    pst = pstp.tile([128, 5, 128], dt.float16, space="PSUM")
                    for t in range(5):
                        nc.tensor.transpose(
                            pst[:, t, :],
                            ur[:, bb * 576 + t * 128: bb * 576 + (t + 1) * 128],
                            ident[:])
                    nc.scalar.copy(xt[:, :, bb * 128:(bb + 1) * 128], pst[:])
                ps = psm.tile([64, BC * 128], f32)
                for t in range(5):
                    nc.tensor.matmul(ps[:], lhsT=w2[:, t * 64:(t + 1) * 64],
                                     rhs=xt[:, t, :], start=(t == 0), stop=(t == 4))
                co = c % OB
                nc.scalar.copy(osb[:, co * BC * 128:(co + 1) * BC * 128], ps[:])
                if co == OB - 1:
                    off0 = (rs + BC * (c + 1)) * 128 - OB * BC * 128
                    nc.sync.dma_start(out_d[:, off0:off0 + OB * BC * 128], osb[:])

            # ---------------- staged pipeline (front-loaded) ----------------
            NSTG = SP // H1
            fronts = []
            for st in range(NSTG):
                rs = st * H1
                OFF = emit_A(rs, H1)
                dwe, mm = emit_B_dense(rs, H1, OFF)
                i2w, w4h2 = emit_C(rs, H1, OFF, dwe, mm)
                fronts.append((rs, i2w, w4h2))
            for rs, i2w, w4h2 in fronts:
                g2s = [emit_D_trig(g, i2w) for g in range(H1 // RC)]
                osb = None
                for c in range(H1 // BC):
                    if c % OB == 0:
                        osb = osp.tile([64, OB * BC * 128], dt.float16)
                    g = c // (RC // BC)
                    cg = c % (RC // BC)
                    ur = emit_D_blend(c, g2s[g], cg, w4h2)
                    emit_D_mm(rs, c, ur, osb)

    nc.compile()
    return nc


def _get_program():
    if "nc" not in _CACHE:
        _CACHE["nc"] = _build_program()
    return _CACHE["nc"]


# ---------------------------------------------------------------------------
# host prep
# ---------------------------------------------------------------------------
def _prep_image(x_img, depth_img):
    """x_img (64,128,128) f32, depth_img (128,128) f32 -> (r2, x_pad)."""
    x_pad = np.pad(x_img, ((0, 0), (1, 1), (1, 1)))
    xp2 = np.pad(x_pad, ((0, 0), (0, 1), (0, 1)))          # (64,131,131)
    xhwc = np.ascontiguousarray(np.transpose(xp2, (1, 2, 0)))  # (131,131,64)
    r2s = np.empty((WP, WP, 64, 4), np.float16)
    r2s[..., 0] = xhwc[:WP, :WP]
    r2s[..., 1] = xhwc[:WP, 1:WP + 1]
    r2s[..., 2] = xhwc[1:WP + 1, :WP]
    r2s[..., 3] = xhwc[1:WP + 1, 1:WP + 1]
    # record layout [c//2, corner, c%2] so both the weight-mul and the
    # corner-pair adds hit the DVE 2x packed mode
    r2s = np.ascontiguousarray(
        r2s.reshape(WP, WP, 32, 2, 4).transpose(0, 1, 2, 4, 3)).reshape(WP, WP, 256)
    r2 = np.zeros((WP2, WP2, 256), np.float16)
    r2[SH:SH + WP, SH:SH + WP] = r2s
    return r2.reshape(NREC, 256), x_pad


def kernel(x, depth, w_p, b_p, w_conv):
    from concourse.bass_utils import run_bass_kernel_spmd

    x = np.asarray(x, np.float32)
    depth = np.asarray(depth, np.float32)
    w_p = np.asarray(w_p, np.float32)
    b_p = np.asarray(b_p, np.float32)
    w_conv = np.asarray(w_conv, np.float32)

    nc = _get_program()

    # weights, shared
    wp_t = np.zeros((65, 9, 18), np.float32)
    for k in range(9):
        wp_t[:64, k, :] = w_p[:, :, k // 3, k % 3].T
    wp_t[64, 4, :] = b_p
    wp_t = wp_t.reshape(65, 162).astype(np.float16)

    W2 = np.transpose(w_conv.reshape(64, 64, 9), (2, 1, 0)).reshape(576, 64)
    W2p = np.zeros((640, 64), np.float32)
    W2p[:576] = W2
    w2_t = np.ascontiguousarray(
        W2p.reshape(5, 128, 64).transpose(1, 0, 2).reshape(128, 320)).astype(np.float16)

    pn_x = np.repeat(np.arange(-1, 2), 3).astype(np.float32)
    pn_y = np.tile(np.arange(-1, 2), 3).astype(np.float32)

    in_maps = []
    per_img = {}
    for img in range(B):
        per_img[img] = _prep_image(x[img], depth[img, 0])
        # padded depth for DET construction
    for core in range(8):
        img, st = divmod(core, 4)
        r0 = st * SP
        r2, x_pad = per_img[img]
        xs = np.empty((65, 34, WP), np.float16)
        xs[:64] = x_pad[:, r0:r0 + 34, :]
        xs[64] = 1.0
        base = np.empty((128, 32, 18), np.float32)
        rows = (r0 + np.arange(32, dtype=np.float32) + 1.0)
        cols = (np.arange(128, dtype=np.float32) + 1.0)
        base[:, :, 0:9] = rows[None, :, None] + pn_x[None, None, :] + SH
        base[:, :, 9:18] = cols[:, None, None] + pn_y[None, None, :] + SH
        dcen = np.ascontiguousarray(depth[img, 0, r0:r0 + 32, :].T)
        # DET: 5 col-shifted clamp-extended depth tiles [j, s(5), t(36)]
        dp = np.pad(depth[img, 0], ((1, 1), (1, 1)))       # (130,130)
        trows = np.clip(r0 - 1 + np.arange(36), 0, H - 1)   # t = row - (r0-1)
        det = np.empty((128, 5, 36), np.float32)
        for si in range(5):
            ccols = np.clip(np.arange(128) + si - 1, 0, W - 1)  # col=j+1+(si-2)
            det[:, si, :] = dp[np.ix_(trows, ccols)].T
        in_maps.append({
            "xs": xs.reshape(65, 34 * WP),
            "r2": r2,
            "det": det.reshape(128, 5 * 36),
            "base": base.reshape(128, 32 * 18),
            "dcen": dcen,
            "wp": wp_t,
            "w2": w2_t,
        })

    res = run_bass_kernel_spmd(nc, in_maps, core_ids=list(range(8)))
    out = np.empty((B, 64, H, W), np.float32)
    for core in range(8):
        img, st = divmod(core, 4)
        out[img, :, st * SP:(st + 1) * SP, :] = \
            res.results[core]["o"].astype(np.float32).reshape(64, SP, W)
    return out


# revision 19
# speedup vs baseline: 1.0034x; 1.0034x over previous
"""Deformable-conv (depth-aware) Trainium2 kernel.

Sharding: pure data parallel — 8 cores = 2 images x 4 H-strips of 32 rows.

Device algorithm per core (strip of 32 rows x 128 cols = 4096 pixels, 9
samples each):
  1. offset conv (PE): off[pix, 18] = sum_k x_slice @ w_p_k   (K=65 incl bias)
  2. pass-1 depth bilinear sampling computed DENSELY (no gather): the
     offsets are < 1 in magnitude, so each sample's 2x2 bilinear footprint
     lies in a 3x3 window around its integer base position.  The depth map
     is host-prepped into 5 column-shifted clamp-extended tiles, and the
     sampling is a separable (3 row-weights x 3 col-weights) accumulation
     of shifted views — all on-chip, zero DMA.  Depth weights dw, m (ACT exp).
  3. off2 = off * dw; pass-2 coords/weights; per-corner weights w4 = m*row*col
  4. dma_gather of 2x2x64ch x-records (fp16, channel-major/corner-minor),
     one DVE mul (weights broadcast over channels) + corner-reduce
  5. DMA-transpose to [(n,c), pix] tiles, PE matmul vs w_conv -> out strip
"""
import numpy as np

B, C, H, W = 2, 64, 128, 128
N = 9
WP = W + 2           # 130 padded width
SH = 4               # coordinate shift: keeps sample coords positive so
                     # int-cast truncation == floor (no correction ops)
WP2 = WP + 2 * SH    # 138 shifted table width
SP = H // 4          # 32 strip rows
NPIX = SP * W        # 4096 pixels per strip
NS = NPIX * N        # 36864 samples per strip
NREC = WP2 * WP2     # shifted-table records
OCLIP = 0.99951171875  # fp16-exact clamp keeping pass-1 window in 3x3

_CACHE = {}


# ---------------------------------------------------------------------------
# device program
# ---------------------------------------------------------------------------
def _build_program():
    import concourse.bacc as bacc
    import concourse.tile as tile
    import concourse.mybir as mybir

    dt = mybir.dt
    Alu = mybir.AluOpType
    Act = mybir.ActivationFunctionType

    nc = bacc.Bacc("TRN2", target_bir_lowering=False, debug=False,
                   enable_asserts=False, num_devices=8)

    xs_d = nc.dram_tensor("xs", [65, 34 * WP], dt.float16, kind="ExternalInput")
    r2_d = nc.dram_tensor("r2", [NREC, 256], dt.float16, kind="ExternalInput")
    det_d = nc.dram_tensor("det", [128, 5 * 36], dt.float32, kind="ExternalInput")
    idsel_d = nc.dram_tensor("idsel", [128, 128], dt.float32, kind="ExternalInput")
    base_d = nc.dram_tensor("base", [128, 32 * 18], dt.float32, kind="ExternalInput")
    dcen_d = nc.dram_tensor("dcen", [128, 32], dt.float32, kind="ExternalInput")
    wp_d = nc.dram_tensor("wp", [65, 9 * 18], dt.float16, kind="ExternalInput")
    w2_d = nc.dram_tensor("w2", [128, 5 * 64], dt.float16, kind="ExternalInput")
    out_d = nc.dram_tensor("o", [64, NPIX], dt.float16, kind="ExternalOutput")

    import os
    H1 = int(os.environ.get('KSTG', '16'))  # rows per pipeline stage
    RC = int(os.environ.get('KRC', '8'))    # rows per gather
    BC = int(os.environ.get('KBC', '2'))    # rows per blend/matmul chunk
    OB = int(os.environ.get('KOB', '4'))    # blend chunks per output store
    KA2 = int(os.environ.get('KA2', '2'))   # add2: 0=DVE 1=Pool 2=alt
    KDP = int(os.environ.get('KDP', '3'))   # dense-pass1: every KDP'th tt op on Pool (0=none)

    with tile.TileContext(nc) as tc:
        with (
            tc.tile_pool(name="const", bufs=1) as cp,
            tc.tile_pool(name="work", bufs=2) as wk,
            tc.tile_pool(name="front", bufs=int(os.environ.get("KFRB", "4"))) as frp,
            tc.tile_pool(name="g2p", bufs=int(os.environ.get("KG2B", "2"))) as g2p,
            tc.tile_pool(name="pstp", bufs=int(os.environ.get("KPST", "2")),
                         space="PSUM") as pstp,
            tc.tile_pool(name="urp", bufs=int(os.environ.get("KURB", "2"))) as urp,
            tc.tile_pool(name="xtp", bufs=int(os.environ.get("KXTB", "2"))) as xtp,
            tc.tile_pool(name="osp", bufs=2) as osp,
            tc.tile_pool(name="psc", bufs=2, space="PSUM") as psc,
            tc.tile_pool(name="psm", bufs=2, space="PSUM") as psm,
            tc.tile_pool(name="psi", bufs=2, space="PSUM") as psi,
        ):
            f32 = dt.float32
            # ---- constants
            xs = cp.tile([65, 34, WP], dt.float16, tag="xs")
            xsv = xs_d[:].rearrange("c (a b) -> c a b", b=WP)
            nc.sync.dma_start(xs[:, 0:18, :], xsv[:, 0:18, :])
            nc.sync.dma_start(xs[:, 18:34, :], xsv[:, 18:34, :])
            det = cp.tile([128, 5, 36], f32, tag="det")
            nc.sync.dma_start(det[:], det_d[:].rearrange("p (a b) -> p a b", b=36))
            base = cp.tile([128, 32, 18], f32, tag="base")
            nc.sync.dma_start(base[:], base_d[:].rearrange("p (a b) -> p a b", b=18))
            dcen = cp.tile([128, 32], f32, tag="dcen")
            nc.sync.dma_start(dcen[:], dcen_d[:])
            wp = cp.tile([65, 9 * 18], dt.float16, tag="wp")
            nc.sync.dma_start(wp[:], wp_d[:])
            w2 = cp.tile([128, 5 * 64], dt.float16, tag="w2")
            nc.sync.dma_start(w2[:], w2_d[:])
            ident = cp.tile([128, 128], dt.float16, tag="ident")
            from concourse.masks import make_identity
            make_identity(nc, ident[:])
            idsel = cp.tile([128, 128], f32, tag="idsel")
            nc.sync.dma_start(idsel[:], idsel_d[:])

            # round-robin engine picker for dense-pass1 tensor_tensor ops
            _dp_ct = [0]

            def dpeng():
                _dp_ct[0] += 1
                if KDP and _dp_ct[0] % KDP == 0:
                    return nc.gpsimd
                return nc.vector

            # pass-2 weight-math engine split: every KCL'th op on Pool
            KCL = int(os.environ.get('KCL', '2'))
            _cl_ct = [0]

            def cleng():
                _cl_ct[0] += 1
                if KCL and _cl_ct[0] % KCL == 0:
                    return nc.gpsimd
                return nc.vector

            def sample_floor(Pc, bound, RR, pool, pfx):
                """floor/clip in SH-shifted coords (pass-2 path, as baseline)."""
                fi = pool.tile([128, RR, 18], dt.int32, tag=pfx + "sm_fi")
                nc.scalar.activation(fi[:], Pc[:], Act.Copy, bias=-0.5)
                f = pool.tile([128, RR, 18], f32, tag=pfx + "sm_f")
                nc.scalar.copy(f[:], fi[:])
                qlt = pool.tile([128, RR, 18], f32, tag=pfx + "sm_qlt")
                nc.vector.tensor_scalar(qlt[:], f[:], float(SH), float(bound - 1 + SH),
                                        Alu.max, Alu.min)
                qrb = pool.tile([128, RR, 18], f32, tag=pfx + "sm_qrb")
                nc.vector.tensor_scalar(qrb[:], f[:], float(SH - 1), float(bound - 2 + SH),
                                        Alu.max, Alu.min)
                nc.scalar.add(qrb[:], qrb[:], 1.0)
                r0 = pool.tile([128, RR, 18], f32, tag=pfx + "sm_r0")
                nc.vector.tensor_scalar(r0[:], qlt[:], float(SH), float(bound - 2 + SH),
                                        Alu.max, Alu.min)
                return r0, qlt, qrb

            def sample_weights(Pc, bound, r0, qlt, qrb, RR, pool, pfx):
                pc = pool.tile([128, RR, 18], f32, tag=pfx + "sm_pc")
                nc.vector.tensor_scalar(pc[:], Pc[:], float(SH), float(bound - 1 + SH),
                                        Alu.max, Alu.min)
                gl = pool.tile([128, RR, 18], f32, tag=pfx + "sm_gl")
                nc.vector.scalar_tensor_tensor(gl[:], qlt[:], 1.0, pc[:], Alu.add, Alu.subtract)
                gr = pool.tile([128, RR, 18], f32, tag=pfx + "sm_gr")
                nc.vector.scalar_tensor_tensor(gr[:], pc[:], 1.0, qrb[:], Alu.add, Alu.subtract)
                eq = pool.tile([128, RR, 18], f32, tag=pfx + "sm_eq")
                wA = pool.tile([128, RR, 18], f32, tag=pfx + "sm_wA")
                wB = pool.tile([128, RR, 18], f32, tag=pfx + "sm_wB")
                tmp = pool.tile([128, RR, 18], f32, tag=pfx + "sm_tmp")
                cleng().tensor_tensor(eq[:], qlt[:], r0[:], Alu.is_equal)
                cleng().tensor_tensor(wA[:], gl[:], eq[:], Alu.mult)
                cleng().tensor_tensor(eq[:], qrb[:], r0[:], Alu.is_equal)
                cleng().tensor_tensor(tmp[:], gr[:], eq[:], Alu.mult)
                cleng().tensor_tensor(wA[:], wA[:], tmp[:], Alu.add)
                nc.vector.scalar_tensor_tensor(
                    eq[:], qlt[:], -1.0, r0[:], Alu.add, Alu.is_equal)
                cleng().tensor_tensor(wB[:], gl[:], eq[:], Alu.mult)
                nc.vector.scalar_tensor_tensor(
                    eq[:], qrb[:], -1.0, r0[:], Alu.add, Alu.is_equal)
                cleng().tensor_tensor(tmp[:], gr[:], eq[:], Alu.mult)
                cleng().tensor_tensor(wB[:], wB[:], tmp[:], Alu.add)
                return wA, wB

            def make_idx(r0, name, RR, pool):
                """Pack per-sample idx into the gather-consumed [16, (n, s)]
                layout via 8 fp32 selector matmuls (PE) instead of 8
                2-byte-granular strided DMAs."""
                NW = RR * 9
                idxf = pool.tile([128, NW], f32, tag=name + "_f")
                nc.vector.scalar_tensor_tensor(
                    idxf[:].rearrange("p (a b) -> p a b", b=9),
                    r0[:, :, 0:9], float(WP2), r0[:, :, 9:18],
                    Alu.mult, Alu.add)
                idxw = pool.tile([128, NW, 8], dt.int16, tag=name + "_w")
                for s0 in range(0, 8, 3):
                    cnt = min(3, 8 - s0)
                    psI = psi.tile([16, 3, NW], f32, space="PSUM")
                    for si in range(cnt):
                        nc.tensor.matmul(
                            psI[:, si, :],
                            lhsT=idsel[:, 16 * (s0 + si):16 * (s0 + si + 1)],
                            rhs=idxf[:], start=True, stop=True)
                    # strided convert: dst (p, n, s), src (p, s, n)
                    nc.vector.tensor_copy(
                        idxw[0:16, :, s0:s0 + cnt],
                        psI[:, 0:cnt, :].transpose([0, 2, 1]))
                nc.sync.dma_start(idxw[16:32, :, :], idxw[0:16, :, :])
                nc.sync.dma_start(idxw[32:64, :, :], idxw[0:32, :, :])
                nc.sync.dma_start(idxw[64:96, :, :], idxw[0:32, :, :])
                nc.sync.dma_start(idxw[96:128, :, :], idxw[0:32, :, :])
                return idxw

            # ---------------- per-stage emission closures ----------------
            def emit_A(rs, nr):
                """offset conv rows [rs, rs+nr) -> OFF [128, nr, 18] (PE)."""
                OFF = wk.tile([128, nr, 18], f32, tag="OFF")
                for bg in range(nr // 4):
                    ps = psc.tile([128, 72], f32)
                    for bb in range(4):
                        b = rs + bg * 4 + bb
                        for k in range(9):
                            drr, dcc = k // 3, k % 3
                            nc.tensor.matmul(
                                ps[:, bb * 18:(bb + 1) * 18],
                                lhsT=xs[:, b + drr, dcc:dcc + 128],
                                rhs=wp[:, k * 18:(k + 1) * 18],
                                start=(k == 0), stop=(k == 8),
                            )
                    nc.scalar.copy(OFF[:, bg * 4:(bg + 1) * 4, :],
                                   ps[:].rearrange("p (a b) -> p a b", b=18))
                return OFF

            def emit_B_dense(rs, nr, OFF):
                """pass-1 depth sampling, dense 3x3 separable form (no DMA).

                Returns dwe, mm [128, nr, 9]."""
                offc = wk.tile([128, nr, 18], f32, tag="b_offc")
                nc.vector.tensor_scalar(offc[:], OFF[:], -OCLIP, OCLIP,
                                        Alu.max, Alu.min)
                P1 = wk.tile([128, nr, 18], f32, tag="b_P1")
                nc.vector.tensor_add(P1[:], offc[:], base[:, rs:rs + nr, :])
                fi = wk.tile([128, nr, 18], dt.int32, tag="b_fi")
                nc.scalar.activation(fi[:], P1[:], Act.Copy, bias=-0.5)
                f = wk.tile([128, nr, 18], f32, tag="b_f")
                nc.scalar.copy(f[:], fi[:])
                q0 = wk.tile([128, nr, 18], f32, tag="b_q0")
                nc.vector.tensor_scalar(q0[:], f[:], float(SH), float(H - 1 + SH),
                                        Alu.max, Alu.min)
                q1c = wk.tile([128, nr, 18], f32, tag="b_q1c")
                nc.vector.tensor_scalar(q1c[:], f[:], float(SH - 1), float(H - 2 + SH),
                                        Alu.max, Alu.min)
                pc = wk.tile([128, nr, 18], f32, tag="b_pc")
                nc.vector.tensor_scalar(pc[:], P1[:], float(SH), float(H - 1 + SH),
                                        Alu.max, Alu.min)
                g0 = wk.tile([128, nr, 18], f32, tag="b_g0")
                nc.vector.scalar_tensor_tensor(g0[:], q0[:], 1.0, pc[:],
                                               Alu.add, Alu.subtract)
                g1 = wk.tile([128, nr, 18], f32, tag="b_g1")
                dpeng().tensor_sub(g1[:], pc[:], q1c[:])
                mA = wk.tile([128, nr, 18], f32, tag="b_mA")
                nc.vector.scalar_tensor_tensor(mA[:], f[:], 1.0, base[:, rs:rs + nr, :],
                                               Alu.add, Alu.is_equal)
                # W3 components: Wm = mA*g0, W0 = g0 + mA*(g1-g0), Wp = g1 - mA*g1
                d = wk.tile([128, nr, 18], f32, tag="b_d")
                dpeng().tensor_sub(d[:], g1[:], g0[:])
                Wm = wk.tile([128, nr, 18], f32, tag="b_Wm")
                dpeng().tensor_mul(Wm[:], mA[:], g0[:])
                t = wk.tile([128, nr, 18], f32, tag="b_t")
                dpeng().tensor_mul(t[:], mA[:], d[:])
                W0 = wk.tile([128, nr, 18], f32, tag="b_W0")
                dpeng().tensor_add(W0[:], g0[:], t[:])
                dpeng().tensor_mul(t[:], mA[:], g1[:])
                Wp = wk.tile([128, nr, 18], f32, tag="b_Wp")
                dpeng().tensor_sub(Wp[:], g1[:], t[:])
                W3 = (Wm, W0, Wp)
                # separable accumulation over the 3x3 window
                V = wk.tile([128, nr, 9], f32, tag="b_V")
                CI = wk.tile([128, nr, 9], f32, tag="b_CI")
                tt = wk.tile([128, nr, 9], f32, tag="b_tt")
                da = det[:]
                for ai in range(3):          # row window offset a'' = ai-1
                    for bi in range(3):      # col window offset b'' = bi-1
                        # DET view: dims (i: stride 1, nr) (dr: stride 1, 3)
                        # (dc: stride 36, 3); offset = bi*36 + rs + ai
                        dv = da.__replace__(
                            offset=da.offset + bi * 36 + rs + ai,
                            ap=type(da.ap)(
                                [[180, 128], [1, nr], [1, 3], [36, 3]]))
                        tgt = CI if bi == 0 else tt
                        dpeng().tensor_tensor(
                            tgt[:].rearrange("p a (u v) -> p a u v", u=3),
                            W3[bi][:, :, 9:18].rearrange("p a (u v) -> p a u v", u=3),
                            dv, Alu.mult)
                        if bi > 0:
                            dpeng().tensor_add(CI[:], CI[:], tt[:])
                    tgt = V if ai == 0 else tt
                    dpeng().tensor_mul(tgt[:], W3[ai][:, :, 0:9], CI[:])
                    if ai > 0:
                        nc.vector.tensor_add(V[:], V[:], tt[:])
                dd = wk.tile([128, nr, 9], f32, tag="b_dd")
                dwe = wk.tile([128, nr, 9], f32, tag="b_dwe")
                mm = wk.tile([128, nr, 9], f32, tag="b_mm")
                nc.vector.tensor_sub(
                    dd[:], dcen[:, rs:rs + nr, None].to_broadcast((128, nr, 9)),
                    V[:])
                nc.scalar.activation(dd[:], dd[:], Act.Abs)
                nc.scalar.activation(dwe[:], dd[:], Act.Exp, scale=-4.0)
                nc.scalar.activation(mm[:], dd[:], Act.Exp, scale=-1.0)
                return dwe, mm

            def emit_C(rs, nr, OFF, dwe, mm):
                NRW = nr * 9
                P2 = wk.tile([128, nr, 18], f32, tag="P2")
                nc.vector.scalar_tensor_tensor(
                    P2[:, :, 0:9], dwe[:], 0.25, OFF[:, :, 0:9], Alu.add, Alu.mult)
                nc.vector.scalar_tensor_tensor(
                    P2[:, :, 9:18], dwe[:], 0.25, OFF[:, :, 9:18], Alu.add, Alu.mult)
                nc.vector.tensor_add(P2[:], P2[:], base[:, rs:rs + nr, :])
                r0_2, qlt2, qrb2 = sample_floor(P2, H + 2, nr, wk, "c")
                idx2w = make_idx(r0_2, "idx2", nr, frp)
                wA2, wB2 = sample_weights(P2, H + 2, r0_2, qlt2, qrb2, nr, wk, "c")
                wTm = wk.tile([128, nr, 9], f32, tag="wTm")
                nc.vector.tensor_mul(wTm[:], wA2[:, :, 0:9], mm[:])
                wBm = wk.tile([128, nr, 9], f32, tag="wBm")
                nc.vector.tensor_mul(wBm[:], wB2[:, :, 0:9], mm[:])
                w4 = wk.tile([128, NRW, 4], f32, tag="w4")
                w4v = w4[:].rearrange("p (a b) c -> p a b c", b=9)
                nc.vector.tensor_mul(w4v[:, :, :, 0], wTm[:], wA2[:, :, 9:18])
                nc.vector.tensor_mul(w4v[:, :, :, 1], wTm[:], wB2[:, :, 9:18])
                nc.vector.tensor_mul(w4v[:, :, :, 2], wBm[:], wA2[:, :, 9:18])
                nc.vector.tensor_mul(w4v[:, :, :, 3], wBm[:], wB2[:, :, 9:18])
                w4h2 = frp.tile([128, NRW, 4, 2], dt.float16, tag="w4h2")
                nc.scalar.copy(
                    w4h2[:], w4[:, :, :, None].to_broadcast((128, NRW, 4, 2)))
                return idx2w, w4h2

            R9 = BC * 9    # blend slots per partition per chunk

            def emit_D_trig(g, idx2w, rcs):
                g2 = g2p.tile([128, RC * 9, 256], dt.float16)
                nc.gpsimd.dma_gather(
                    out_ap=g2[:, 0:rcs * 9, :], in_ap=r2_d[:],
                    idxs_ap=idx2w[:, rcs * 9 * g:rcs * 9 * (g + 1), :],
                    num_idxs=1152 * rcs, num_idxs_reg=1152 * rcs, elem_size=256,
                    single_packet=False)
                return g2

            def emit_D_blend(c, g2, cg, w4h2):
                # blend in place: g2 is dead after the corner adds
                g2s = g2[:, R9 * cg:R9 * (cg + 1), :]
                u4 = g2s.rearrange("p a (h k l) -> p a h k l", k=4, l=2)
                nc.vector.tensor_tensor(
                    u4, u4,
                    w4h2[:, R9 * c:R9 * (c + 1), None, :, :].to_broadcast(
                        (128, R9, 32, 4, 2)),
                    Alu.mult)
                u4v = g2s.rearrange("p a (h k l) -> p (a h) k l", k=4, l=2)
                nc.vector.tensor_tensor(u4v[:, :, 0:2, :], u4v[:, :, 0:2, :],
                                        u4v[:, :, 2:4, :], Alu.add)
                ur = urp.tile([128, BC * 576 + 64], dt.float16)
                nc.vector.memset(ur[:, BC * 576:BC * 576 + 64], 0.0)
                urv = ur[:, 0:BC * 576].rearrange("p (a l) -> p a l", l=2)
                eng = (nc.gpsimd if (KA2 == 1 or (KA2 == 2 and c % 2 == 0))
                       else nc.vector)
                eng.tensor_tensor(urv, u4v[:, :, 0, :], u4v[:, :, 1, :],
                                  Alu.add)
                return ur

            KXC = int(os.environ.get('KXC', '0'))  # xt copy: 0=Act 1=alt Act/Pool

            def emit_D_mm(rs, c, ur, osb, co, nob):
                xt = xtp.tile([128, 5, BC * 128], dt.float16)
                for bb in range(BC):
                    # 5 transposes land in one PSUM bank -> single copy
                    pst = pstp.tile([128, 5, 128], dt.float16, space="PSUM")
                    for t in range(5):
                        nc.tensor.transpose(
                            pst[:, t, :],
                            ur[:, bb * 576 + t * 128: bb * 576 + (t + 1) * 128],
                            ident[:])
                    eng = nc.gpsimd if (KXC and (c * BC + bb) % 2 == 0) else nc.scalar
                    if eng is nc.gpsimd:
                        eng.tensor_copy(xt[:, :, bb * 128:(bb + 1) * 128], pst[:])
                    else:
                        eng.copy(xt[:, :, bb * 128:(bb + 1) * 128], pst[:])
                ps = psm.tile([64, BC * 128], f32)
                for t in range(5):
                    nc.tensor.matmul(ps[:], lhsT=w2[:, t * 64:(t + 1) * 64],
                                     rhs=xt[:, t, :], start=(t == 0), stop=(t == 4))
                nc.scalar.copy(osb[:, co * BC * 128:(co + 1) * BC * 128], ps[:])
                if co == nob - 1:
                    off0 = (rs + BC * (c + 1)) * 128 - nob * BC * 128
                    nc.sync.dma_start(out_d[:, off0:off0 + nob * BC * 128], osb[:])

            # ---------------- staged pipeline (front-loaded) ----------------
            plan = [int(x) for x in
                    os.environ.get('KPLAN', str(H1)).split(',')]
            while sum(plan) < SP:
                plan.append(plan[-1])
            assert sum(plan) == SP, plan
            fronts = []
            rs = 0
            for nr in plan:
                OFF = emit_A(rs, nr)
                dwe, mm = emit_B_dense(rs, nr, OFF)
                i2w, w4h2 = emit_C(rs, nr, OFF, dwe, mm)
                fronts.append((rs, nr, i2w, w4h2))
                rs += nr
            for rs, nr, i2w, w4h2 in fronts:
                rcs = min(RC, nr)
                g2s = [emit_D_trig(g, i2w, rcs) for g in range(nr // rcs)]
                osb = None
                nchunk = nr // BC
                for c in range(nchunk):
                    co = c % OB
                    nob = min(OB, nchunk - (c - co))
                    if co == 0:
                        osb = osp.tile([64, OB * BC * 128], dt.float16)
                    g = c // (rcs // BC)
                    cg = c % (rcs // BC)
                    ur = emit_D_blend(c, g2s[g], cg, w4h2)
                    emit_D_mm(rs, c, ur, osb, co, nob)

    nc.compile()
    return nc


def _get_program():
    if "nc" not in _CACHE:
        _CACHE["nc"] = _build_program()
    return _CACHE["nc"]


# ---------------------------------------------------------------------------
# host prep
# ---------------------------------------------------------------------------
def _prep_image(x_img, depth_img):
    """x_img (64,128,128) f32, depth_img (128,128) f32 -> (r2, x_pad)."""
    x_pad = np.pad(x_img, ((0, 0), (1, 1), (1, 1)))
    xp2 = np.pad(x_pad, ((0, 0), (0, 1), (0, 1)))          # (64,131,131)
    xhwc = np.ascontiguousarray(np.transpose(xp2, (1, 2, 0)))  # (131,131,64)
    r2s = np.empty((WP, WP, 64, 4), np.float16)
    r2s[..., 0] = xhwc[:WP, :WP]
    r2s[..., 1] = xhwc[:WP, 1:WP + 1]
    r2s[..., 2] = xhwc[1:WP + 1, :WP]
    r2s[..., 3] = xhwc[1:WP + 1, 1:WP + 1]
    # record layout [c//2, corner, c%2] so both the weight-mul and the
    # corner-pair adds hit the DVE 2x packed mode
    r2s = np.ascontiguousarray(
        r2s.reshape(WP, WP, 32, 2, 4).transpose(0, 1, 2, 4, 3)).reshape(WP, WP, 256)
    r2 = np.zeros((WP2, WP2, 256), np.float16)
    r2[SH:SH + WP, SH:SH + WP] = r2s
    return r2.reshape(NREC, 256), x_pad


def kernel(x, depth, w_p, b_p, w_conv):
    from concourse.bass_utils import run_bass_kernel_spmd

    x = np.asarray(x, np.float32)
    depth = np.asarray(depth, np.float32)
    w_p = np.asarray(w_p, np.float32)
    b_p = np.asarray(b_p, np.float32)
    w_conv = np.asarray(w_conv, np.float32)

    nc = _get_program()

    # weights, shared
    wp_t = np.zeros((65, 9, 18), np.float32)
    for k in range(9):
        wp_t[:64, k, :] = w_p[:, :, k // 3, k % 3].T
    wp_t[64, 4, :] = b_p
    wp_t = wp_t.reshape(65, 162).astype(np.float16)

    W2 = np.transpose(w_conv.reshape(64, 64, 9), (2, 1, 0)).reshape(576, 64)
    W2p = np.zeros((640, 64), np.float32)
    W2p[:576] = W2
    w2_t = np.ascontiguousarray(
        W2p.reshape(5, 128, 64).transpose(1, 0, 2).reshape(128, 320)).astype(np.float16)

    pn_x = np.repeat(np.arange(-1, 2), 3).astype(np.float32)
    pn_y = np.tile(np.arange(-1, 2), 3).astype(np.float32)

    in_maps = []
    per_img = {}
    for img in range(B):
        per_img[img] = _prep_image(x[img], depth[img, 0])
        # padded depth for DET construction
    for core in range(8):
        img, st = divmod(core, 4)
        r0 = st * SP
        r2, x_pad = per_img[img]
        xs = np.empty((65, 34, WP), np.float16)
        xs[:64] = x_pad[:, r0:r0 + 34, :]
        xs[64] = 1.0
        base = np.empty((128, 32, 18), np.float32)
        rows = (r0 + np.arange(32, dtype=np.float32) + 1.0)
        cols = (np.arange(128, dtype=np.float32) + 1.0)
        base[:, :, 0:9] = rows[None, :, None] + pn_x[None, None, :] + SH
        base[:, :, 9:18] = cols[:, None, None] + pn_y[None, None, :] + SH
        dcen = np.ascontiguousarray(depth[img, 0, r0:r0 + 32, :].T)
        # DET: 5 col-shifted clamp-extended depth tiles [j, s(5), t(36)]
        dp = np.pad(depth[img, 0], ((1, 1), (1, 1)))       # (130,130)
        trows = np.clip(r0 - 1 + np.arange(36), 0, H - 1)   # t = row - (r0-1)
        det = np.empty((128, 5, 36), np.float32)
        for si in range(5):
            ccols = np.clip(np.arange(128) + si - 1, 0, W - 1)  # col=j+1+(si-2)
            det[:, si, :] = dp[np.ix_(trows, ccols)].T
        in_maps.append({
            "xs": xs.reshape(65, 34 * WP),
            "r2": r2,
            "det": det.reshape(128, 5 * 36),
            "base": base.reshape(128, 32 * 18),
            "dcen": dcen,
            "wp": wp_t,
            "w2": w2_t,
            "idsel": np.eye(128, dtype=np.float32),
        })

    res = run_bass_kernel_spmd(nc, in_maps, core_ids=list(range(8)))
    out = np.empty((B, 64, H, W), np.float32)
    for core in range(8):
        img, st = divmod(core, 4)
        out[img, :, st * SP:(st + 1) * SP, :] = \
            res.results[core]["o"].astype(np.float32).reshape(64, SP, W)
    return out


# revision 22
# speedup vs baseline: 1.0054x; 1.0020x over previous
"""Deformable-conv (depth-aware) Trainium2 kernel.

Sharding: pure data parallel — 8 cores = 2 images x 4 H-strips of 32 rows.

Device algorithm per core (strip of 32 rows x 128 cols = 4096 pixels, 9
samples each):
  1. offset conv (PE): off[pix, 18] = sum_k x_slice @ w_p_k   (K=65 incl bias)
  2. pass-1 depth bilinear sampling computed DENSELY (no gather): the
     offsets are < 1 in magnitude, so each sample's 2x2 bilinear footprint
     lies in a 3x3 window around its integer base position.  The depth map
     is host-prepped into 5 column-shifted clamp-extended tiles, and the
     sampling is a separable (3 row-weights x 3 col-weights) accumulation
     of shifted views — all on-chip, zero DMA.  Depth weights dw, m (ACT exp).
  3. off2 = off * dw; pass-2 coords/weights; per-corner weights w4 = m*row*col
  4. dma_gather of 2x2x64ch x-records (fp16, channel-major/corner-minor),
     one DVE mul (weights broadcast over channels) + corner-reduce
  5. DMA-transpose to [(n,c), pix] tiles, PE matmul vs w_conv -> out strip
"""
import numpy as np

B, C, H, W = 2, 64, 128, 128
N = 9
WP = W + 2           # 130 padded width
SH = 4               # coordinate shift: keeps sample coords positive so
                     # int-cast truncation == floor (no correction ops)
WP2 = WP + 2 * SH    # 138 shifted table width
SP = H // 4          # 32 strip rows
NPIX = SP * W        # 4096 pixels per strip
NS = NPIX * N        # 36864 samples per strip
NREC = WP2 * WP2     # shifted-table records
OCLIP = 0.99951171875  # fp16-exact clamp keeping pass-1 window in 3x3

_CACHE = {}


# ---------------------------------------------------------------------------
# device program
# ---------------------------------------------------------------------------
def _build_program():
    import concourse.bacc as bacc
    import concourse.tile as tile
    import concourse.mybir as mybir

    dt = mybir.dt
    Alu = mybir.AluOpType
    Act = mybir.ActivationFunctionType

    nc = bacc.Bacc("TRN2", target_bir_lowering=False, debug=False,
                   enable_asserts=False, num_devices=8)

    xs_d = nc.dram_tensor("xs", [65, 34 * WP], dt.float16, kind="ExternalInput")
    r2_d = nc.dram_tensor("r2", [NREC, 256], dt.float16, kind="ExternalInput")
    det_d = nc.dram_tensor("det", [128, 5 * 36], dt.float32, kind="ExternalInput")
    idsel_d = nc.dram_tensor("idsel", [128, 128], dt.float32, kind="ExternalInput")
    base_d = nc.dram_tensor("base", [128, 32 * 18], dt.float32, kind="ExternalInput")
    dcen_d = nc.dram_tensor("dcen", [128, 32], dt.float32, kind="ExternalInput")
    wp_d = nc.dram_tensor("wp", [65, 9 * 18], dt.float16, kind="ExternalInput")
    w2_d = nc.dram_tensor("w2", [128, 5 * 64], dt.float16, kind="ExternalInput")
    out_d = nc.dram_tensor("o", [64, NPIX], dt.float16, kind="ExternalOutput")

    import os
    H1 = int(os.environ.get('KSTG', '16'))  # rows per pipeline stage
    RC = int(os.environ.get('KRC', '8'))    # rows per gather
    BC = int(os.environ.get('KBC', '2'))    # rows per blend/matmul chunk
    OB = int(os.environ.get('KOB', '4'))    # blend chunks per output store
    KA2 = int(os.environ.get('KA2', '2'))   # add2: 0=DVE 1=Pool 2=alt
    KDP = int(os.environ.get('KDP', '3'))   # dense-pass1: every KDP'th tt op on Pool (0=none)

    with tile.TileContext(nc) as tc:
        with (
            tc.tile_pool(name="const", bufs=1) as cp,
            tc.tile_pool(name="work", bufs=2) as wk,
            tc.tile_pool(name="front", bufs=int(os.environ.get("KFRB", "4"))) as frp,
            tc.tile_pool(name="g2p", bufs=int(os.environ.get("KG2B", "2"))) as g2p,
            tc.tile_pool(name="pstp", bufs=int(os.environ.get("KPST", "2")),
                         space="PSUM") as pstp,
            tc.tile_pool(name="urp", bufs=int(os.environ.get("KURB", "2"))) as urp,
            tc.tile_pool(name="xtp", bufs=int(os.environ.get("KXTB", "2"))) as xtp,
            tc.tile_pool(name="osp", bufs=2) as osp,
            tc.tile_pool(name="psc", bufs=int(os.environ.get("KPSC", "1")),
                         space="PSUM") as psc,
            tc.tile_pool(name="psm", bufs=2, space="PSUM") as psm,
            tc.tile_pool(name="psi", bufs=int(os.environ.get("KPSI", "1")),
                         space="PSUM") as psi,
        ):
            f32 = dt.float32
            # ---- constants
            xs = cp.tile([65, 34, WP], dt.float16, tag="xs")
            xsv = xs_d[:].rearrange("c (a b) -> c a b", b=WP)
            nc.sync.dma_start(xs[:, 0:18, :], xsv[:, 0:18, :])
            nc.sync.dma_start(xs[:, 18:34, :], xsv[:, 18:34, :])
            det = cp.tile([128, 5, 36], f32, tag="det")
            nc.sync.dma_start(det[:], det_d[:].rearrange("p (a b) -> p a b", b=36))
            base = cp.tile([128, 32, 18], f32, tag="base")
            nc.sync.dma_start(base[:], base_d[:].rearrange("p (a b) -> p a b", b=18))
            dcen = cp.tile([128, 32], f32, tag="dcen")
            nc.sync.dma_start(dcen[:], dcen_d[:])
            wp = cp.tile([65, 9 * 18], dt.float16, tag="wp")
            nc.sync.dma_start(wp[:], wp_d[:])
            w2 = cp.tile([128, 5 * 64], dt.float16, tag="w2")
            nc.sync.dma_start(w2[:], w2_d[:])
            ident = cp.tile([128, 128], dt.float16, tag="ident")
            from concourse.masks import make_identity
            make_identity(nc, ident[:])
            idsel = cp.tile([128, 128], f32, tag="idsel")
            nc.sync.dma_start(idsel[:], idsel_d[:])

            # round-robin engine picker for dense-pass1 tensor_tensor ops
            _dp_ct = [0]

            def dpeng():
                _dp_ct[0] += 1
                if KDP and _dp_ct[0] % KDP == 0:
                    return nc.gpsimd
                return nc.vector

            # pass-2 weight-math engine split: every KCL'th op on Pool
            KCL = int(os.environ.get('KCL', '2'))
            _cl_ct = [0]

            def cleng():
                _cl_ct[0] += 1
                if KCL and _cl_ct[0] % KCL == 0:
                    return nc.gpsimd
                return nc.vector

            def sample_floor(Pc, bound, RR, pool, pfx):
                """floor/clip in SH-shifted coords (pass-2 path, as baseline)."""
                fi = pool.tile([128, RR, 18], dt.int32, tag=pfx + "sm_fi")
                nc.scalar.activation(fi[:], Pc[:], Act.Copy, bias=-0.5)
                f = pool.tile([128, RR, 18], f32, tag=pfx + "sm_f")
                nc.scalar.copy(f[:], fi[:])
                qlt = pool.tile([128, RR, 18], f32, tag=pfx + "sm_qlt")
                nc.vector.tensor_scalar(qlt[:], f[:], float(SH), float(bound - 1 + SH),
                                        Alu.max, Alu.min)
                qrb = pool.tile([128, RR, 18], f32, tag=pfx + "sm_qrb")
                nc.vector.tensor_scalar(qrb[:], f[:], float(SH - 1), float(bound - 2 + SH),
                                        Alu.max, Alu.min)
                nc.scalar.add(qrb[:], qrb[:], 1.0)
                r0 = pool.tile([128, RR, 18], f32, tag=pfx + "sm_r0")
                nc.vector.tensor_scalar(r0[:], qlt[:], float(SH), float(bound - 2 + SH),
                                        Alu.max, Alu.min)
                return r0, qlt, qrb

            def sample_weights(Pc, bound, r0, qlt, qrb, RR, pool, pfx):
                pc = pool.tile([128, RR, 18], f32, tag=pfx + "sm_pc")
                nc.vector.tensor_scalar(pc[:], Pc[:], float(SH), float(bound - 1 + SH),
                                        Alu.max, Alu.min)
                gl = pool.tile([128, RR, 18], f32, tag=pfx + "sm_gl")
                nc.vector.scalar_tensor_tensor(gl[:], qlt[:], 1.0, pc[:], Alu.add, Alu.subtract)
                gr = pool.tile([128, RR, 18], f32, tag=pfx + "sm_gr")
                nc.vector.scalar_tensor_tensor(gr[:], pc[:], 1.0, qrb[:], Alu.add, Alu.subtract)
                eq = pool.tile([128, RR, 18], f32, tag=pfx + "sm_eq")
                wA = pool.tile([128, RR, 18], f32, tag=pfx + "sm_wA")
                wB = pool.tile([128, RR, 18], f32, tag=pfx + "sm_wB")
                tmp = pool.tile([128, RR, 18], f32, tag=pfx + "sm_tmp")
                cleng().tensor_tensor(eq[:], qlt[:], r0[:], Alu.is_equal)
                cleng().tensor_tensor(wA[:], gl[:], eq[:], Alu.mult)
                cleng().tensor_tensor(eq[:], qrb[:], r0[:], Alu.is_equal)
                cleng().tensor_tensor(tmp[:], gr[:], eq[:], Alu.mult)
                cleng().tensor_tensor(wA[:], wA[:], tmp[:], Alu.add)
                nc.vector.scalar_tensor_tensor(
                    eq[:], qlt[:], -1.0, r0[:], Alu.add, Alu.is_equal)
                cleng().tensor_tensor(wB[:], gl[:], eq[:], Alu.mult)
                nc.vector.scalar_tensor_tensor(
                    eq[:], qrb[:], -1.0, r0[:], Alu.add, Alu.is_equal)
                cleng().tensor_tensor(tmp[:], gr[:], eq[:], Alu.mult)
                cleng().tensor_tensor(wB[:], wB[:], tmp[:], Alu.add)
                return wA, wB

            def make_idx(r0, name, RR, pool):
                """Pack per-sample idx into the gather-consumed [16, (n, s)]
                layout via 8 fp32 selector matmuls (PE) instead of 8
                2-byte-granular strided DMAs."""
                NW = RR * 9
                idxf = pool.tile([128, NW], f32, tag=name + "_f")
                nc.vector.scalar_tensor_tensor(
                    idxf[:].rearrange("p (a b) -> p a b", b=9),
                    r0[:, :, 0:9], float(WP2), r0[:, :, 9:18],
                    Alu.mult, Alu.add)
                idxw = pool.tile([128, NW, 8], dt.int16, tag=name + "_w")
                for s0 in range(0, 8, 3):
                    cnt = min(3, 8 - s0)
                    psI = psi.tile([16, 3, NW], f32, space="PSUM")
                    for si in range(cnt):
                        nc.tensor.matmul(
                            psI[:, si, :],
                            lhsT=idsel[:, 16 * (s0 + si):16 * (s0 + si + 1)],
                            rhs=idxf[:], start=True, stop=True)
                    # strided convert: dst (p, n, s), src (p, s, n)
                    nc.vector.tensor_copy(
                        idxw[0:16, :, s0:s0 + cnt],
                        psI[:, 0:cnt, :].transpose([0, 2, 1]))
                nc.sync.dma_start(idxw[16:32, :, :], idxw[0:16, :, :])
                nc.sync.dma_start(idxw[32:64, :, :], idxw[0:32, :, :])
                nc.sync.dma_start(idxw[64:96, :, :], idxw[0:32, :, :])
                nc.sync.dma_start(idxw[96:128, :, :], idxw[0:32, :, :])
                return idxw

            # ---------------- per-stage emission closures ----------------
            def emit_A(rs, nr):
                """offset conv rows [rs, rs+nr) -> OFF [128, nr, 18] (PE).

                Computed transposed ([18, 4rows*128pix] accumulating the 9
                taps in one PSUM bank -> 9 matmuls per 4-row group instead of
                36), then PE-transposed back to [j, (row, 18)]."""
                OFF = wk.tile([128, nr, 18], f32, tag="OFF")
                for bg in range(nr // 4):
                    b = rs + bg * 4
                    ps = psc.tile([18, 512], f32, tag="psA")
                    for k in range(9):
                        drr, dcc = k // 3, k % 3
                        nc.tensor.matmul(
                            ps[:],
                            lhsT=wp[:, k * 18:(k + 1) * 18],
                            rhs=xs[:, b + drr:b + drr + 4, dcc:dcc + 128],
                            start=(k == 0), stop=(k == 8),
                        )
                    so = wk.tile([18, 4, 128], f32, tag="soA")
                    nc.scalar.copy(so[:], ps[:].rearrange("p (a b) -> p a b", b=128))
                    psT = psc.tile([128, 4, 18], f32, tag="psAT")
                    for r in range(4):
                        nc.tensor.transpose(psT[:, r, :], so[:, r, :],
                                            idsel[0:18, 0:18])
                    nc.scalar.copy(OFF[:, bg * 4:(bg + 1) * 4, :], psT[:])
                return OFF

            def emit_B_dense(rs, nr, OFF):
                """pass-1 depth sampling, dense 3x3 separable form (no DMA).

                Returns dwe, mm [128, nr, 9]."""
                offc = wk.tile([128, nr, 18], f32, tag="b_offc")
                nc.vector.tensor_scalar(offc[:], OFF[:], -OCLIP, OCLIP,
                                        Alu.max, Alu.min)
                P1 = wk.tile([128, nr, 18], f32, tag="b_P1")
                nc.vector.tensor_add(P1[:], offc[:], base[:, rs:rs + nr, :])
                fi = wk.tile([128, nr, 18], dt.int32, tag="b_fi")
                nc.scalar.activation(fi[:], P1[:], Act.Copy, bias=-0.5)
                f = wk.tile([128, nr, 18], f32, tag="b_f")
                nc.scalar.copy(f[:], fi[:])
                q0 = wk.tile([128, nr, 18], f32, tag="b_q0")
                nc.vector.tensor_scalar(q0[:], f[:], float(SH), float(H - 1 + SH),
                                        Alu.max, Alu.min)
                q1c = wk.tile([128, nr, 18], f32, tag="b_q1c")
                nc.vector.tensor_scalar(q1c[:], f[:], float(SH - 1), float(H - 2 + SH),
                                        Alu.max, Alu.min)
                pc = wk.tile([128, nr, 18], f32, tag="b_pc")
                nc.vector.tensor_scalar(pc[:], P1[:], float(SH), float(H - 1 + SH),
                                        Alu.max, Alu.min)
                g0 = wk.tile([128, nr, 18], f32, tag="b_g0")
                nc.vector.scalar_tensor_tensor(g0[:], q0[:], 1.0, pc[:],
                                               Alu.add, Alu.subtract)
                g1 = wk.tile([128, nr, 18], f32, tag="b_g1")
                dpeng().tensor_sub(g1[:], pc[:], q1c[:])
                mA = wk.tile([128, nr, 18], f32, tag="b_mA")
                nc.vector.scalar_tensor_tensor(mA[:], f[:], 1.0, base[:, rs:rs + nr, :],
                                               Alu.add, Alu.is_equal)
                # W3 components: Wm = mA*g0, W0 = g0 + mA*(g1-g0), Wp = g1 - mA*g1
                d = wk.tile([128, nr, 18], f32, tag="b_d")
                dpeng().tensor_sub(d[:], g1[:], g0[:])
                Wm = wk.tile([128, nr, 18], f32, tag="b_Wm")
                dpeng().tensor_mul(Wm[:], mA[:], g0[:])
                t = wk.tile([128, nr, 18], f32, tag="b_t")
                dpeng().tensor_mul(t[:], mA[:], d[:])
                W0 = wk.tile([128, nr, 18], f32, tag="b_W0")
                dpeng().tensor_add(W0[:], g0[:], t[:])
                dpeng().tensor_mul(t[:], mA[:], g1[:])
                Wp = wk.tile([128, nr, 18], f32, tag="b_Wp")
                dpeng().tensor_sub(Wp[:], g1[:], t[:])
                W3 = (Wm, W0, Wp)
                # separable accumulation over the 3x3 window
                V = wk.tile([128, nr, 9], f32, tag="b_V")
                CI = wk.tile([128, nr, 9], f32, tag="b_CI")
                tt = wk.tile([128, nr, 9], f32, tag="b_tt")
                da = det[:]
                for ai in range(3):          # row window offset a'' = ai-1
                    for bi in range(3):      # col window offset b'' = bi-1
                        # DET view: dims (i: stride 1, nr) (dr: stride 1, 3)
                        # (dc: stride 36, 3); offset = bi*36 + rs + ai
                        dv = da.__replace__(
                            offset=da.offset + bi * 36 + rs + ai,
                            ap=type(da.ap)(
                                [[180, 128], [1, nr], [1, 3], [36, 3]]))
                        tgt = CI if bi == 0 else tt
                        dpeng().tensor_tensor(
                            tgt[:].rearrange("p a (u v) -> p a u v", u=3),
                            W3[bi][:, :, 9:18].rearrange("p a (u v) -> p a u v", u=3),
                            dv, Alu.mult)
                        if bi > 0:
                            dpeng().tensor_add(CI[:], CI[:], tt[:])
                    tgt = V if ai == 0 else tt
                    dpeng().tensor_mul(tgt[:], W3[ai][:, :, 0:9], CI[:])
                    if ai > 0:
                        nc.vector.tensor_add(V[:], V[:], tt[:])
                dd = wk.tile([128, nr, 9], f32, tag="b_dd")
                dwe = wk.tile([128, nr, 9], f32, tag="b_dwe")
                mm = wk.tile([128, nr, 9], f32, tag="b_mm")
                nc.vector.tensor_sub(
                    dd[:], dcen[:, rs:rs + nr, None].to_broadcast((128, nr, 9)),
                    V[:])
                nc.scalar.activation(dd[:], dd[:], Act.Abs)
                nc.scalar.activation(dwe[:], dd[:], Act.Exp, scale=-4.0)
                nc.scalar.activation(mm[:], dd[:], Act.Exp, scale=-1.0)
                return dwe, mm

            def emit_C(rs, nr, OFF, dwe, mm):
                NRW = nr * 9
                P2 = wk.tile([128, nr, 18], f32, tag="P2")
                nc.vector.scalar_tensor_tensor(
                    P2[:, :, 0:9], dwe[:], 0.25, OFF[:, :, 0:9], Alu.add, Alu.mult)
                nc.vector.scalar_tensor_tensor(
                    P2[:, :, 9:18], dwe[:], 0.25, OFF[:, :, 9:18], Alu.add, Alu.mult)
                nc.vector.tensor_add(P2[:], P2[:], base[:, rs:rs + nr, :])
                r0_2, qlt2, qrb2 = sample_floor(P2, H + 2, nr, wk, "c")
                idx2w = make_idx(r0_2, "idx2", nr, frp)
                wA2, wB2 = sample_weights(P2, H + 2, r0_2, qlt2, qrb2, nr, wk, "c")
                wTm = wk.tile([128, nr, 9], f32, tag="wTm")
                nc.vector.tensor_mul(wTm[:], wA2[:, :, 0:9], mm[:])
                wBm = wk.tile([128, nr, 9], f32, tag="wBm")
                nc.vector.tensor_mul(wBm[:], wB2[:, :, 0:9], mm[:])
                w4 = wk.tile([128, NRW, 4], f32, tag="w4")
                w4v = w4[:].rearrange("p (a b) c -> p a b c", b=9)
                nc.vector.tensor_mul(w4v[:, :, :, 0], wTm[:], wA2[:, :, 9:18])
                nc.vector.tensor_mul(w4v[:, :, :, 1], wTm[:], wB2[:, :, 9:18])
                nc.vector.tensor_mul(w4v[:, :, :, 2], wBm[:], wA2[:, :, 9:18])
                nc.vector.tensor_mul(w4v[:, :, :, 3], wBm[:], wB2[:, :, 9:18])
                w4h2 = frp.tile([128, NRW, 4, 2], dt.float16, tag="w4h2")
                nc.scalar.copy(
                    w4h2[:], w4[:, :, :, None].to_broadcast((128, NRW, 4, 2)))
                return idx2w, w4h2

            R9 = BC * 9    # blend slots per partition per chunk

            def emit_D_trig(g, idx2w, rcs):
                g2 = g2p.tile([128, RC * 9, 256], dt.float16)
                nc.gpsimd.dma_gather(
                    out_ap=g2[:, 0:rcs * 9, :], in_ap=r2_d[:],
                    idxs_ap=idx2w[:, rcs * 9 * g:rcs * 9 * (g + 1), :],
                    num_idxs=1152 * rcs, num_idxs_reg=1152 * rcs, elem_size=256,
                    single_packet=False)
                return g2

            def emit_D_blend(c, g2, cg, w4h2):
                # blend in place: g2 is dead after the corner adds
                g2s = g2[:, R9 * cg:R9 * (cg + 1), :]
                u4 = g2s.rearrange("p a (h k l) -> p a h k l", k=4, l=2)
                nc.vector.tensor_tensor(
                    u4, u4,
                    w4h2[:, R9 * c:R9 * (c + 1), None, :, :].to_broadcast(
                        (128, R9, 32, 4, 2)),
                    Alu.mult)
                u4v = g2s.rearrange("p a (h k l) -> p (a h) k l", k=4, l=2)
                nc.vector.tensor_tensor(u4v[:, :, 0:2, :], u4v[:, :, 0:2, :],
                                        u4v[:, :, 2:4, :], Alu.add)
                ur = urp.tile([128, BC * 576 + 64], dt.float16)
                nc.vector.memset(ur[:, BC * 576:BC * 576 + 64], 0.0)
                urv = ur[:, 0:BC * 576].rearrange("p (a l) -> p a l", l=2)
                eng = (nc.gpsimd if (KA2 == 1 or (KA2 == 2 and c % 2 == 0))
                       else nc.vector)
                eng.tensor_tensor(urv, u4v[:, :, 0, :], u4v[:, :, 1, :],
                                  Alu.add)
                return ur

            KXC = int(os.environ.get('KXC', '0'))  # xt copy: 0=Act 1=alt Act/Pool

            def emit_D_mm(rs, c, ur, osb, co, nob):
                xt = xtp.tile([128, 5, BC * 128], dt.float16)
                for bb in range(BC):
                    # 5 transposes land in one PSUM bank -> single copy
                    pst = pstp.tile([128, 5, 128], dt.float16, space="PSUM")
                    for t in range(5):
                        nc.tensor.transpose(
                            pst[:, t, :],
                            ur[:, bb * 576 + t * 128: bb * 576 + (t + 1) * 128],
                            ident[:])
                    eng = nc.gpsimd if (KXC and (c * BC + bb) % 2 == 0) else nc.scalar
                    if eng is nc.gpsimd:
                        eng.tensor_copy(xt[:, :, bb * 128:(bb + 1) * 128], pst[:])
                    else:
                        eng.copy(xt[:, :, bb * 128:(bb + 1) * 128], pst[:])
                ps = psm.tile([64, BC * 128], f32)
                for t in range(5):
                    nc.tensor.matmul(ps[:], lhsT=w2[:, t * 64:(t + 1) * 64],
                                     rhs=xt[:, t, :], start=(t == 0), stop=(t == 4))
                nc.scalar.copy(osb[:, co * BC * 128:(co + 1) * BC * 128], ps[:])
                if co == nob - 1:
                    off0 = (rs + BC * (c + 1)) * 128 - nob * BC * 128
                    nc.sync.dma_start(out_d[:, off0:off0 + nob * BC * 128],
                                      osb[:, 0:nob * BC * 128])

            # ---------------- staged pipeline (front-loaded) ----------------
            plan = [int(x) for x in
                    os.environ.get('KPLAN', str(H1)).split(',')]
            while sum(plan) < SP:
                plan.append(plan[-1])
            assert sum(plan) == SP, plan
            fronts = []
            rs = 0
            for nr in plan:
                OFF = emit_A(rs, nr)
                dwe, mm = emit_B_dense(rs, nr, OFF)
                i2w, w4h2 = emit_C(rs, nr, OFF, dwe, mm)
                fronts.append((rs, nr, i2w, w4h2))
                rs += nr
            for rs, nr, i2w, w4h2 in fronts:
                rcs = min(RC, nr)
                g2s = [emit_D_trig(g, i2w, rcs) for g in range(nr // rcs)]
                osb = None
                nchunk = nr // BC
                for c in range(nchunk):
                    co = c % OB
                    nob = min(OB, nchunk - (c - co))
                    if co == 0:
                        osb = osp.tile([64, OB * BC * 128], dt.float16)
                    g = c // (rcs // BC)
                    cg = c % (rcs // BC)
                    ur = emit_D_blend(c, g2s[g], cg, w4h2)
                    emit_D_mm(rs, c, ur, osb, co, nob)

    nc.compile()
    return nc


def _get_program():
    if "nc" not in _CACHE:
        _CACHE["nc"] = _build_program()
    return _CACHE["nc"]


# ---------------------------------------------------------------------------
# host prep
# ---------------------------------------------------------------------------
def _prep_image(x_img, depth_img):
    """x_img (64,128,128) f32, depth_img (128,128) f32 -> (r2, x_pad)."""
    x_pad = np.pad(x_img, ((0, 0), (1, 1), (1, 1)))
    xp2 = np.pad(x_pad, ((0, 0), (0, 1), (0, 1)))          # (64,131,131)
    xhwc = np.ascontiguousarray(np.transpose(xp2, (1, 2, 0)))  # (131,131,64)
    r2s = np.empty((WP, WP, 64, 4), np.float16)
    r2s[..., 0] = xhwc[:WP, :WP]
    r2s[..., 1] = xhwc[:WP, 1:WP + 1]
    r2s[..., 2] = xhwc[1:WP + 1, :WP]
    r2s[..., 3] = xhwc[1:WP + 1, 1:WP + 1]
    # record layout [c//2, corner, c%2] so both the weight-mul and the
    # corner-pair adds hit the DVE 2x packed mode
    r2s = np.ascontiguousarray(
        r2s.reshape(WP, WP, 32, 2, 4).transpose(0, 1, 2, 4, 3)).reshape(WP, WP, 256)
    r2 = np.zeros((WP2, WP2, 256), np.float16)
    r2[SH:SH + WP, SH:SH + WP] = r2s
    return r2.reshape(NREC, 256), x_pad


def kernel(x, depth, w_p, b_p, w_conv):
    from concourse.bass_utils import run_bass_kernel_spmd

    x = np.asarray(x, np.float32)
    depth = np.asarray(depth, np.float32)
    w_p = np.asarray(w_p, np.float32)
    b_p = np.asarray(b_p, np.float32)
    w_conv = np.asarray(w_conv, np.float32)

    nc = _get_program()

    # weights, shared
    wp_t = np.zeros((65, 9, 18), np.float32)
    for k in range(9):
        wp_t[:64, k, :] = w_p[:, :, k // 3, k % 3].T
    wp_t[64, 4, :] = b_p
    wp_t = wp_t.reshape(65, 162).astype(np.float16)

    W2 = np.transpose(w_conv.reshape(64, 64, 9), (2, 1, 0)).reshape(576, 64)
    W2p = np.zeros((640, 64), np.float32)
    W2p[:576] = W2
    w2_t = np.ascontiguousarray(
        W2p.reshape(5, 128, 64).transpose(1, 0, 2).reshape(128, 320)).astype(np.float16)

    pn_x = np.repeat(np.arange(-1, 2), 3).astype(np.float32)
    pn_y = np.tile(np.arange(-1, 2), 3).astype(np.float32)

    in_maps = []
    per_img = {}
    for img in range(B):
        per_img[img] = _prep_image(x[img], depth[img, 0])
        # padded depth for DET construction
    for core in range(8):
        img, st = divmod(core, 4)
        r0 = st * SP
        r2, x_pad = per_img[img]
        xs = np.empty((65, 34, WP), np.float16)
        xs[:64] = x_pad[:, r0:r0 + 34, :]
        xs[64] = 1.0
        base = np.empty((128, 32, 18), np.float32)
        rows = (r0 + np.arange(32, dtype=np.float32) + 1.0)
        cols = (np.arange(128, dtype=np.float32) + 1.0)
        base[:, :, 0:9] = rows[None, :, None] + pn_x[None, None, :] + SH
        base[:, :, 9:18] = cols[:, None, None] + pn_y[None, None, :] + SH
        dcen = np.ascontiguousarray(depth[img, 0, r0:r0 + 32, :].T)
        # DET: 5 col-shifted clamp-extended depth tiles [j, s(5), t(36)]
        dp = np.pad(depth[img, 0], ((1, 1), (1, 1)))       # (130,130)
        trows = np.clip(r0 - 1 + np.arange(36), 0, H - 1)   # t = row - (r0-1)
        det = np.empty((128, 5, 36), np.float32)
        for si in range(5):
            ccols = np.clip(np.arange(128) + si - 1, 0, W - 1)  # col=j+1+(si-2)
            det[:, si, :] = dp[np.ix_(trows, ccols)].T
        in_maps.append({
            "xs": xs.reshape(65, 34 * WP),
            "r2": r2,
            "det": det.reshape(128, 5 * 36),
            "base": base.reshape(128, 32 * 18),
            "dcen": dcen,
            "wp": wp_t,
            "w2": w2_t,
            "idsel": np.eye(128, dtype=np.float32),
        })

    res = run_bass_kernel_spmd(nc, in_maps, core_ids=list(range(8)))
    out = np.empty((B, 64, H, W), np.float32)
    for core in range(8):
        img, st = divmod(core, 4)
        out[img, :, st * SP:(st + 1) * SP, :] = \
            res.results[core]["o"].astype(np.float32).reshape(64, SP, W)
    return out


# revision 23
# speedup vs baseline: 1.0724x; 1.0666x over previous
"""Deformable-conv (depth-aware) Trainium2 kernel.

Sharding: pure data parallel — 8 cores = 2 images x 4 H-strips of 32 rows.

Device algorithm per core (strip of 32 rows x 128 cols = 4096 pixels, 9
samples each):
  1. offset conv (PE): off[pix, 18] = sum_k x_slice @ w_p_k   (K=65 incl bias)
  2. pass-1 depth bilinear sampling computed DENSELY (no gather): the
     offsets are < 1 in magnitude, so each sample's 2x2 bilinear footprint
     lies in a 3x3 window around its integer base position.  The depth map
     is host-prepped into 5 column-shifted clamp-extended tiles, and the
     sampling is a separable (3 row-weights x 3 col-weights) accumulation
     of shifted views — all on-chip, zero DMA.  Depth weights dw, m (ACT exp).
  3. off2 = off * dw; pass-2 coords/weights; per-corner weights w4 = m*row*col
  4. dma_gather of 2x2x64ch x-records (fp16, channel-major/corner-minor),
     one DVE mul (weights broadcast over channels) + corner-reduce
  5. DMA-transpose to [(n,c), pix] tiles, PE matmul vs w_conv -> out strip
"""
import numpy as np

B, C, H, W = 2, 64, 128, 128
N = 9
WP = W + 2           # 130 padded width
SH = 4               # coordinate shift: keeps sample coords positive so
                     # int-cast truncation == floor (no correction ops)
WP2 = WP + 2 * SH    # 138 shifted table width
SP = H // 4          # 32 strip rows
NPIX = SP * W        # 4096 pixels per strip
NS = NPIX * N        # 36864 samples per strip
NREC = WP2 * WP2     # shifted-table records
OCLIP = 0.99951171875  # fp16-exact clamp keeping pass-1 window in 3x3

_CACHE = {}


# ---------------------------------------------------------------------------
# device program
# ---------------------------------------------------------------------------
def _build_program():
    import concourse.bacc as bacc
    import concourse.tile as tile
    import concourse.mybir as mybir

    dt = mybir.dt
    Alu = mybir.AluOpType
    Act = mybir.ActivationFunctionType

    nc = bacc.Bacc("TRN2", target_bir_lowering=False, debug=False,
                   enable_asserts=False, num_devices=8)

    xs_d = nc.dram_tensor("xs", [65, 34 * WP], dt.float16, kind="ExternalInput")
    r2_d = nc.dram_tensor("r2", [NREC, 256], dt.float16, kind="ExternalInput")
    det_d = nc.dram_tensor("det", [128, 5 * 36], dt.float32, kind="ExternalInput")
    idsel_d = nc.dram_tensor("idsel", [128, 128], dt.float32, kind="ExternalInput")
    base_d = nc.dram_tensor("base", [128, 32 * 18], dt.float32, kind="ExternalInput")
    dcen_d = nc.dram_tensor("dcen", [128, 32], dt.float32, kind="ExternalInput")
    wp_d = nc.dram_tensor("wp", [65, 9 * 18], dt.float16, kind="ExternalInput")
    w2_d = nc.dram_tensor("w2", [128, 5 * 64], dt.float16, kind="ExternalInput")
    out_d = nc.dram_tensor("o", [64, NPIX], dt.float16, kind="ExternalOutput")

    import os
    H1 = int(os.environ.get('KSTG', '16'))  # rows per pipeline stage
    RC = int(os.environ.get('KRC', '8'))    # rows per gather
    BC = int(os.environ.get('KBC', '2'))    # rows per blend/matmul chunk
    OB = int(os.environ.get('KOB', '4'))    # blend chunks per output store
    KA2 = int(os.environ.get('KA2', '2'))   # add2: 0=DVE 1=Pool 2=alt
    KDP = int(os.environ.get('KDP', '3'))   # dense-pass1: every KDP'th tt op on Pool (0=none)

    with tile.TileContext(nc) as tc:
        with (
            tc.tile_pool(name="const", bufs=1) as cp,
            tc.tile_pool(name="work", bufs=2) as wk,
            tc.tile_pool(name="front", bufs=int(os.environ.get("KFRB", "4"))) as frp,
            tc.tile_pool(name="g2p", bufs=int(os.environ.get("KG2B", "2"))) as g2p,
            tc.tile_pool(name="pstp", bufs=int(os.environ.get("KPST", "2")),
                         space="PSUM") as pstp,
            tc.tile_pool(name="urp", bufs=int(os.environ.get("KURB", "2"))) as urp,
            tc.tile_pool(name="xtp", bufs=int(os.environ.get("KXTB", "2"))) as xtp,
            tc.tile_pool(name="osp", bufs=2) as osp,
            tc.tile_pool(name="psc", bufs=int(os.environ.get("KPSC", "1")),
                         space="PSUM") as psc,
            tc.tile_pool(name="psm", bufs=2, space="PSUM") as psm,
            tc.tile_pool(name="psi", bufs=int(os.environ.get("KPSI", "1")),
                         space="PSUM") as psi,
        ):
            f32 = dt.float32
            # ---- constants
            xs = cp.tile([65, 34, WP], dt.float16, tag="xs")
            xsv = xs_d[:].rearrange("c (a b) -> c a b", b=WP)
            nc.sync.dma_start(xs[:, 0:18, :], xsv[:, 0:18, :])
            nc.sync.dma_start(xs[:, 18:34, :], xsv[:, 18:34, :])
            det = cp.tile([128, 5, 36], f32, tag="det")
            nc.sync.dma_start(det[:], det_d[:].rearrange("p (a b) -> p a b", b=36))
            base = cp.tile([128, 32, 18], f32, tag="base")
            nc.sync.dma_start(base[:], base_d[:].rearrange("p (a b) -> p a b", b=18))
            dcen = cp.tile([128, 32], f32, tag="dcen")
            nc.sync.dma_start(dcen[:], dcen_d[:])
            wp = cp.tile([65, 9 * 18], dt.float16, tag="wp")
            nc.sync.dma_start(wp[:], wp_d[:])
            w2 = cp.tile([128, 5 * 64], dt.float16, tag="w2")
            nc.sync.dma_start(w2[:], w2_d[:])
            ident = cp.tile([128, 128], dt.float16, tag="ident")
            from concourse.masks import make_identity
            make_identity(nc, ident[:])
            idsel = cp.tile([128, 128], f32, tag="idsel")
            nc.sync.dma_start(idsel[:], idsel_d[:])

            # round-robin engine picker for dense-pass1 tensor_tensor ops
            _dp_ct = [0]

            def dpeng():
                _dp_ct[0] += 1
                if KDP and _dp_ct[0] % KDP == 0:
                    return nc.gpsimd
                return nc.vector

            # pass-2 weight-math engine split: every KCL'th op on Pool
            KCL = int(os.environ.get('KCL', '2'))
            _cl_ct = [0]

            def cleng():
                _cl_ct[0] += 1
                if KCL and _cl_ct[0] % KCL == 0:
                    return nc.gpsimd
                return nc.vector

            def sample_floor(Pc, bound, RR, pool, pfx):
                """floor/clip in SH-shifted coords (pass-2 path, as baseline)."""
                fi = pool.tile([128, RR, 18], dt.int32, tag=pfx + "sm_fi")
                nc.scalar.activation(fi[:], Pc[:], Act.Copy, bias=-0.5)
                f = pool.tile([128, RR, 18], f32, tag=pfx + "sm_f")
                nc.scalar.copy(f[:], fi[:])
                qlt = pool.tile([128, RR, 18], f32, tag=pfx + "sm_qlt")
                nc.vector.tensor_scalar(qlt[:], f[:], float(SH), float(bound - 1 + SH),
                                        Alu.max, Alu.min)
                qrb = pool.tile([128, RR, 18], f32, tag=pfx + "sm_qrb")
                nc.vector.tensor_scalar(qrb[:], f[:], float(SH - 1), float(bound - 2 + SH),
                                        Alu.max, Alu.min)
                nc.scalar.add(qrb[:], qrb[:], 1.0)
                r0 = pool.tile([128, RR, 18], f32, tag=pfx + "sm_r0")
                nc.vector.tensor_scalar(r0[:], qlt[:], float(SH), float(bound - 2 + SH),
                                        Alu.max, Alu.min)
                return r0, qlt, qrb

            def sample_weights(Pc, bound, r0, qlt, qrb, RR, pool, pfx):
                pc = pool.tile([128, RR, 18], f32, tag=pfx + "sm_pc")
                nc.vector.tensor_scalar(pc[:], Pc[:], float(SH), float(bound - 1 + SH),
                                        Alu.max, Alu.min)
                gl = pool.tile([128, RR, 18], f32, tag=pfx + "sm_gl")
                nc.vector.scalar_tensor_tensor(gl[:], qlt[:], 1.0, pc[:], Alu.add, Alu.subtract)
                gr = pool.tile([128, RR, 18], f32, tag=pfx + "sm_gr")
                nc.vector.scalar_tensor_tensor(gr[:], pc[:], 1.0, qrb[:], Alu.add, Alu.subtract)
                eq = pool.tile([128, RR, 18], f32, tag=pfx + "sm_eq")
                wA = pool.tile([128, RR, 18], f32, tag=pfx + "sm_wA")
                wB = pool.tile([128, RR, 18], f32, tag=pfx + "sm_wB")
                tmp = pool.tile([128, RR, 18], f32, tag=pfx + "sm_tmp")
                cleng().tensor_tensor(eq[:], qlt[:], r0[:], Alu.is_equal)
                cleng().tensor_tensor(wA[:], gl[:], eq[:], Alu.mult)
                cleng().tensor_tensor(eq[:], qrb[:], r0[:], Alu.is_equal)
                cleng().tensor_tensor(tmp[:], gr[:], eq[:], Alu.mult)
                cleng().tensor_tensor(wA[:], wA[:], tmp[:], Alu.add)
                nc.vector.scalar_tensor_tensor(
                    eq[:], qlt[:], -1.0, r0[:], Alu.add, Alu.is_equal)
                cleng().tensor_tensor(wB[:], gl[:], eq[:], Alu.mult)
                nc.vector.scalar_tensor_tensor(
                    eq[:], qrb[:], -1.0, r0[:], Alu.add, Alu.is_equal)
                cleng().tensor_tensor(tmp[:], gr[:], eq[:], Alu.mult)
                cleng().tensor_tensor(wB[:], wB[:], tmp[:], Alu.add)
                return wA, wB

            def make_idx(r0, name, RR, pool):
                """Pack per-sample idx into the gather-consumed [16, (n, s)]
                layout via 8 fp32 selector matmuls (PE) instead of 8
                2-byte-granular strided DMAs."""
                NW = RR * 9
                idxf = pool.tile([128, NW], f32, tag=name + "_f")
                nc.vector.scalar_tensor_tensor(
                    idxf[:].rearrange("p (a b) -> p a b", b=9),
                    r0[:, :, 0:9], float(WP2), r0[:, :, 9:18],
                    Alu.mult, Alu.add)
                idxw = pool.tile([128, NW, 8], dt.int16, tag=name + "_w")
                for s0 in range(0, 8, 3):
                    cnt = min(3, 8 - s0)
                    psI = psi.tile([16, 3, NW], f32, space="PSUM")
                    for si in range(cnt):
                        nc.tensor.matmul(
                            psI[:, si, :],
                            lhsT=idsel[:, 16 * (s0 + si):16 * (s0 + si + 1)],
                            rhs=idxf[:], start=True, stop=True)
                    # strided convert: dst (p, n, s), src (p, s, n)
                    nc.vector.tensor_copy(
                        idxw[0:16, :, s0:s0 + cnt],
                        psI[:, 0:cnt, :].transpose([0, 2, 1]))
                nc.sync.dma_start(idxw[16:32, :, :], idxw[0:16, :, :])
                nc.sync.dma_start(idxw[32:64, :, :], idxw[0:32, :, :])
                nc.sync.dma_start(idxw[64:96, :, :], idxw[0:32, :, :])
                nc.sync.dma_start(idxw[96:128, :, :], idxw[0:32, :, :])
                return idxw

            # ---------------- per-stage emission closures ----------------
            def emit_A(rs, nr):
                """offset conv rows [rs, rs+nr) -> OFF [128, nr, 18] (PE)."""
                OFF = wk.tile([128, nr, 18], f32, tag="OFF")
                for bg in range(nr // 4):
                    ps = psc.tile([128, 72], f32, tag="psA")
                    for bb in range(4):
                        b = rs + bg * 4 + bb
                        for k in range(9):
                            drr, dcc = k // 3, k % 3
                            nc.tensor.matmul(
                                ps[:, bb * 18:(bb + 1) * 18],
                                lhsT=xs[:, b + drr, dcc:dcc + 128],
                                rhs=wp[:, k * 18:(k + 1) * 18],
                                start=(k == 0), stop=(k == 8),
                            )
                    nc.scalar.copy(OFF[:, bg * 4:(bg + 1) * 4, :],
                                   ps[:].rearrange("p (a b) -> p a b", b=18))
                return OFF

            def emit_B_dense(rs, nr, OFF):
                """pass-1 depth sampling, dense 3x3 separable form (no DMA).

                Returns dwe, mm [128, nr, 9]."""
                offc = wk.tile([128, nr, 18], f32, tag="b_offc")
                nc.vector.tensor_scalar(offc[:], OFF[:], -OCLIP, OCLIP,
                                        Alu.max, Alu.min)
                P1 = wk.tile([128, nr, 18], f32, tag="b_P1")
                nc.vector.tensor_add(P1[:], offc[:], base[:, rs:rs + nr, :])
                fi = wk.tile([128, nr, 18], dt.int32, tag="b_fi")
                nc.scalar.activation(fi[:], P1[:], Act.Copy, bias=-0.5)
                f = wk.tile([128, nr, 18], f32, tag="b_f")
                nc.scalar.copy(f[:], fi[:])
                q0 = wk.tile([128, nr, 18], f32, tag="b_q0")
                nc.vector.tensor_scalar(q0[:], f[:], float(SH), float(H - 1 + SH),
                                        Alu.max, Alu.min)
                q1c = wk.tile([128, nr, 18], f32, tag="b_q1c")
                nc.vector.tensor_scalar(q1c[:], f[:], float(SH - 1), float(H - 2 + SH),
                                        Alu.max, Alu.min)
                pc = wk.tile([128, nr, 18], f32, tag="b_pc")
                nc.vector.tensor_scalar(pc[:], P1[:], float(SH), float(H - 1 + SH),
                                        Alu.max, Alu.min)
                g0 = wk.tile([128, nr, 18], f32, tag="b_g0")
                nc.vector.scalar_tensor_tensor(g0[:], q0[:], 1.0, pc[:],
                                               Alu.add, Alu.subtract)
                g1 = wk.tile([128, nr, 18], f32, tag="b_g1")
                dpeng().tensor_sub(g1[:], pc[:], q1c[:])
                mA = wk.tile([128, nr, 18], f32, tag="b_mA")
                nc.vector.scalar_tensor_tensor(mA[:], f[:], 1.0, base[:, rs:rs + nr, :],
                                               Alu.add, Alu.is_equal)
                # W3 components: Wm = mA*g0, W0 = g0 + mA*(g1-g0), Wp = g1 - mA*g1
                d = wk.tile([128, nr, 18], f32, tag="b_d")
                dpeng().tensor_sub(d[:], g1[:], g0[:])
                Wm = wk.tile([128, nr, 18], f32, tag="b_Wm")
                dpeng().tensor_mul(Wm[:], mA[:], g0[:])
                t = wk.tile([128, nr, 18], f32, tag="b_t")
                dpeng().tensor_mul(t[:], mA[:], d[:])
                W0 = wk.tile([128, nr, 18], f32, tag="b_W0")
                dpeng().tensor_add(W0[:], g0[:], t[:])
                dpeng().tensor_mul(t[:], mA[:], g1[:])
                Wp = wk.tile([128, nr, 18], f32, tag="b_Wp")
                dpeng().tensor_sub(Wp[:], g1[:], t[:])
                W3 = (Wm, W0, Wp)
                # separable accumulation over the 3x3 window
                V = wk.tile([128, nr, 9], f32, tag="b_V")
                CI = wk.tile([128, nr, 9], f32, tag="b_CI")
                tt = wk.tile([128, nr, 9], f32, tag="b_tt")
                da = det[:]
                for ai in range(3):          # row window offset a'' = ai-1
                    for bi in range(3):      # col window offset b'' = bi-1
                        # DET view: dims (i: stride 1, nr) (dr: stride 1, 3)
                        # (dc: stride 36, 3); offset = bi*36 + rs + ai
                        dv = da.__replace__(
                            offset=da.offset + bi * 36 + rs + ai,
                            ap=type(da.ap)(
                                [[180, 128], [1, nr], [1, 3], [36, 3]]))
                        tgt = CI if bi == 0 else tt
                        dpeng().tensor_tensor(
                            tgt[:].rearrange("p a (u v) -> p a u v", u=3),
                            W3[bi][:, :, 9:18].rearrange("p a (u v) -> p a u v", u=3),
                            dv, Alu.mult)
                        if bi > 0:
                            dpeng().tensor_add(CI[:], CI[:], tt[:])
                    tgt = V if ai == 0 else tt
                    dpeng().tensor_mul(tgt[:], W3[ai][:, :, 0:9], CI[:])
                    if ai > 0:
                        nc.vector.tensor_add(V[:], V[:], tt[:])
                dd = wk.tile([128, nr, 9], f32, tag="b_dd")
                dwe = wk.tile([128, nr, 9], f32, tag="b_dwe")
                mm = wk.tile([128, nr, 9], f32, tag="b_mm")
                nc.vector.tensor_sub(
                    dd[:], dcen[:, rs:rs + nr, None].to_broadcast((128, nr, 9)),
                    V[:])
                nc.scalar.activation(dd[:], dd[:], Act.Abs)
                nc.scalar.activation(dwe[:], dd[:], Act.Exp, scale=-4.0)
                nc.scalar.activation(mm[:], dd[:], Act.Exp, scale=-1.0)
                return dwe, mm

            def emit_C(rs, nr, OFF, dwe, mm):
                NRW = nr * 9
                P2 = wk.tile([128, nr, 18], f32, tag="P2")
                nc.vector.scalar_tensor_tensor(
                    P2[:, :, 0:9], dwe[:], 0.25, OFF[:, :, 0:9], Alu.add, Alu.mult)
                nc.vector.scalar_tensor_tensor(
                    P2[:, :, 9:18], dwe[:], 0.25, OFF[:, :, 9:18], Alu.add, Alu.mult)
                nc.vector.tensor_add(P2[:], P2[:], base[:, rs:rs + nr, :])
                r0_2, qlt2, qrb2 = sample_floor(P2, H + 2, nr, wk, "c")
                idx2w = make_idx(r0_2, "idx2", nr, frp)
                wA2, wB2 = sample_weights(P2, H + 2, r0_2, qlt2, qrb2, nr, wk, "c")
                wTm = wk.tile([128, nr, 9], f32, tag="wTm")
                nc.vector.tensor_mul(wTm[:], wA2[:, :, 0:9], mm[:])
                wBm = wk.tile([128, nr, 9], f32, tag="wBm")
                nc.vector.tensor_mul(wBm[:], wB2[:, :, 0:9], mm[:])
                w4 = wk.tile([128, NRW, 4], f32, tag="w4")
                w4v = w4[:].rearrange("p (a b) c -> p a b c", b=9)
                nc.vector.tensor_mul(w4v[:, :, :, 0], wTm[:], wA2[:, :, 9:18])
                nc.vector.tensor_mul(w4v[:, :, :, 1], wTm[:], wB2[:, :, 9:18])
                nc.vector.tensor_mul(w4v[:, :, :, 2], wBm[:], wA2[:, :, 9:18])
                nc.vector.tensor_mul(w4v[:, :, :, 3], wBm[:], wB2[:, :, 9:18])
                w4h2 = frp.tile([128, NRW, 4, 2], dt.float16, tag="w4h2")
                nc.scalar.copy(
                    w4h2[:], w4[:, :, :, None].to_broadcast((128, NRW, 4, 2)))
                return idx2w, w4h2

            R9 = BC * 9    # blend slots per partition per chunk

            def emit_D_trig(g, idx2w, rcs):
                g2 = g2p.tile([128, RC * 9, 256], dt.float16)
                nc.gpsimd.dma_gather(
                    out_ap=g2[:, 0:rcs * 9, :], in_ap=r2_d[:],
                    idxs_ap=idx2w[:, rcs * 9 * g:rcs * 9 * (g + 1), :],
                    num_idxs=1152 * rcs, num_idxs_reg=1152 * rcs, elem_size=256,
                    single_packet=False)
                return g2

            def emit_D_blend(c, g2, cg, w4h2):
                # blend in place: g2 is dead after the corner adds
                g2s = g2[:, R9 * cg:R9 * (cg + 1), :]
                u4 = g2s.rearrange("p a (h k l) -> p a h k l", k=4, l=2)
                nc.vector.tensor_tensor(
                    u4, u4,
                    w4h2[:, R9 * c:R9 * (c + 1), None, :, :].to_broadcast(
                        (128, R9, 32, 4, 2)),
                    Alu.mult)
                u4v = g2s.rearrange("p a (h k l) -> p (a h) k l", k=4, l=2)
                nc.vector.tensor_tensor(u4v[:, :, 0:2, :], u4v[:, :, 0:2, :],
                                        u4v[:, :, 2:4, :], Alu.add)
                ur = urp.tile([128, BC * 576 + 64], dt.float16)
                nc.vector.memset(ur[:, BC * 576:BC * 576 + 64], 0.0)
                urv = ur[:, 0:BC * 576].rearrange("p (a l) -> p a l", l=2)
                eng = (nc.gpsimd if (KA2 == 1 or (KA2 == 2 and c % 2 == 0))
                       else nc.vector)
                eng.tensor_tensor(urv, u4v[:, :, 0, :], u4v[:, :, 1, :],
                                  Alu.add)
                return ur

            KXC = int(os.environ.get('KXC', '0'))  # xt copy: 0=Act 1=alt Act/Pool

            def emit_D_mm(rs, c, ur, osb, co, nob):
                xt = xtp.tile([128, 5, BC * 128], dt.float16)
                for bb in range(BC):
                    # 5 transposes land in one PSUM bank -> single copy
                    pst = pstp.tile([128, 5, 128], dt.float16, space="PSUM")
                    for t in range(5):
                        nc.tensor.transpose(
                            pst[:, t, :],
                            ur[:, bb * 576 + t * 128: bb * 576 + (t + 1) * 128],
                            ident[:])
                    eng = nc.gpsimd if (KXC and (c * BC + bb) % 2 == 0) else nc.scalar
                    if eng is nc.gpsimd:
                        eng.tensor_copy(xt[:, :, bb * 128:(bb + 1) * 128], pst[:])
                    else:
                        eng.copy(xt[:, :, bb * 128:(bb + 1) * 128], pst[:])
                ps = psm.tile([64, BC * 128], f32)
                for t in range(5):
                    nc.tensor.matmul(ps[:], lhsT=w2[:, t * 64:(t + 1) * 64],
                                     rhs=xt[:, t, :], start=(t == 0), stop=(t == 4))
                nc.scalar.copy(osb[:, co * BC * 128:(co + 1) * BC * 128], ps[:])
                if co == nob - 1:
                    off0 = (rs + BC * (c + 1)) * 128 - nob * BC * 128
                    nc.sync.dma_start(out_d[:, off0:off0 + nob * BC * 128],
                                      osb[:, 0:nob * BC * 128])

            # ---------------- staged pipeline (front-loaded) ----------------
            plan = [int(x) for x in
                    os.environ.get('KPLAN', str(H1)).split(',')]
            while sum(plan) < SP:
                plan.append(plan[-1])
            assert sum(plan) == SP, plan
            fronts = []
            rs = 0
            for nr in plan:
                OFF = emit_A(rs, nr)
                dwe, mm = emit_B_dense(rs, nr, OFF)
                i2w, w4h2 = emit_C(rs, nr, OFF, dwe, mm)
                fronts.append((rs, nr, i2w, w4h2))
                rs += nr
            for rs, nr, i2w, w4h2 in fronts:
                rcs = min(RC, nr)
                g2s = [emit_D_trig(g, i2w, rcs) for g in range(nr // rcs)]
                osb = None
                nchunk = nr // BC
                for c in range(nchunk):
                    co = c % OB
                    nob = min(OB, nchunk - (c - co))
                    if co == 0:
                        osb = osp.tile([64, OB * BC * 128], dt.float16)
                    g = c // (rcs // BC)
                    cg = c % (rcs // BC)
                    ur = emit_D_blend(c, g2s[g], cg, w4h2)
                    emit_D_mm(rs, c, ur, osb, co, nob)

    nc.compile()
    return nc


def _get_program():
    if "nc" not in _CACHE:
        _CACHE["nc"] = _build_program()
    return _CACHE["nc"]


# ---------------------------------------------------------------------------
# host prep
# ---------------------------------------------------------------------------
def _prep_image(x_img, depth_img):
    """x_img (64,128,128) f32, depth_img (128,128) f32 -> (r2, x_pad)."""
    x_pad = np.pad(x_img, ((0, 0), (1, 1), (1, 1)))
    xp2 = np.pad(x_pad, ((0, 0), (0, 1), (0, 1)))          # (64,131,131)
    xhwc = np.ascontiguousarray(np.transpose(xp2, (1, 2, 0)))  # (131,131,64)
    r2s = np.empty((WP, WP, 64, 4), np.float16)
    r2s[..., 0] = xhwc[:WP, :WP]
    r2s[..., 1] = xhwc[:WP, 1:WP + 1]
    r2s[..., 2] = xhwc[1:WP + 1, :WP]
    r2s[..., 3] = xhwc[1:WP + 1, 1:WP + 1]
    # record layout [c//2, corner, c%2] so both the weight-mul and the
    # corner-pair adds hit the DVE 2x packed mode
    r2s = np.ascontiguousarray(
        r2s.reshape(WP, WP, 32, 2, 4).transpose(0, 1, 2, 4, 3)).reshape(WP, WP, 256)
    r2 = np.zeros((WP2, WP2, 256), np.float16)
    r2[SH:SH + WP, SH:SH + WP] = r2s
    return r2.reshape(NREC, 256), x_pad


def kernel(x, depth, w_p, b_p, w_conv):
    from concourse.bass_utils import run_bass_kernel_spmd

    x = np.asarray(x, np.float32)
    depth = np.asarray(depth, np.float32)
    w_p = np.asarray(w_p, np.float32)
    b_p = np.asarray(b_p, np.float32)
    w_conv = np.asarray(w_conv, np.float32)

    nc = _get_program()

    # weights, shared
    wp_t = np.zeros((65, 9, 18), np.float32)
    for k in range(9):
        wp_t[:64, k, :] = w_p[:, :, k // 3, k % 3].T
    wp_t[64, 4, :] = b_p
    wp_t = wp_t.reshape(65, 162).astype(np.float16)

    W2 = np.transpose(w_conv.reshape(64, 64, 9), (2, 1, 0)).reshape(576, 64)
    W2p = np.zeros((640, 64), np.float32)
    W2p[:576] = W2
    w2_t = np.ascontiguousarray(
        W2p.reshape(5, 128, 64).transpose(1, 0, 2).reshape(128, 320)).astype(np.float16)

    pn_x = np.repeat(np.arange(-1, 2), 3).astype(np.float32)
    pn_y = np.tile(np.arange(-1, 2), 3).astype(np.float32)

    in_maps = []
    per_img = {}
    for img in range(B):
        per_img[img] = _prep_image(x[img], depth[img, 0])
        # padded depth for DET construction
    for core in range(8):
        img, st = divmod(core, 4)
        r0 = st * SP
        r2, x_pad = per_img[img]
        xs = np.empty((65, 34, WP), np.float16)
        xs[:64] = x_pad[:, r0:r0 + 34, :]
        xs[64] = 1.0
        base = np.empty((128, 32, 18), np.float32)
        rows = (r0 + np.arange(32, dtype=np.float32) + 1.0)
        cols = (np.arange(128, dtype=np.float32) + 1.0)
        base[:, :, 0:9] = rows[None, :, None] + pn_x[None, None, :] + SH
        base[:, :, 9:18] = cols[:, None, None] + pn_y[None, None, :] + SH
        dcen = np.ascontiguousarray(depth[img, 0, r0:r0 + 32, :].T)
        # DET: 5 col-shifted clamp-extended depth tiles [j, s(5), t(36)]
        dp = np.pad(depth[img, 0], ((1, 1), (1, 1)))       # (130,130)
        trows = np.clip(r0 - 1 + np.arange(36), 0, H - 1)   # t = row - (r0-1)
        det = np.empty((128, 5, 36), np.float32)
        for si in range(5):
            ccols = np.clip(np.arange(128) + si - 1, 0, W - 1)  # col=j+1+(si-2)
            det[:, si, :] = dp[np.ix_(trows, ccols)].T
        in_maps.append({
            "xs": xs.reshape(65, 34 * WP),
            "r2": r2,
            "det": det.reshape(128, 5 * 36),
            "base": base.reshape(128, 32 * 18),
            "dcen": dcen,
            "wp": wp_t,
            "w2": w2_t,
            "idsel": np.eye(128, dtype=np.float32),
        })

    res = run_bass_kernel_spmd(nc, in_maps, core_ids=list(range(8)))
    out = np.empty((B, 64, H, W), np.float32)
    for core in range(8):
        img, st = divmod(core, 4)
        out[img, :, st * SP:(st + 1) * SP, :] = \
            res.results[core]["o"].astype(np.float32).reshape(64, SP, W)
    return out
